# revision 20
# baseline (speedup 1.0000x reference)
"""Trainium2 Bass kernel for GQA attention (nn_Attention_34832184770944).

Sharding: tensor-parallel across heads on 8 cores. Core m gets KV head m and
Q heads 4m..4m+3: wq/wk/wv sharded column-wise, wo row-wise. Each core
computes a full-shape bf16 partial output; the host sums the 8 partials in
fp32.

Device kernel design (v2, restructured from the 478us baseline):
  - QKV projection computed e-major (lhsT = weight tile, rhs = xT chunk of
    512 tokens) so Q and K come out pre-transposed; no PE transpose pass.
  - Host permutes wq/wk columns within each head (even indices first) so
    RoPE becomes NeoX rotate-half: partition-shift DVE copies + full-width
    bf16 tensor_tensor ops per chunk.
  - V is projected e-major then PE-transposed to token-major; each 128-col
    V block is [vT (64) | ones (64)], so the PV matmul emits softmax
    denominators REPLICATED on PSUM partitions 64:128 at zero extra cost
    (matmul time is set by rhs streaming, not output rows).
  - Normalize fused out of PSUM: reciprocal_approx_fast on rows 64:128 then
    one tensor_mul (PSUM rows 0:64 x rinv -> oT bf16). No DRAM round trip,
    no 1-partition reciprocal, no broadcast matmul, no separate oU copy.
  - Scores per (key tile, head pair): two concurrent row-group matmuls
    (heads at PE row strips 0/64 via KR duplicated into both halves) write
    one [128,1024] fp32 2-bank PSUM tile; ONE batched exp per pair tile
    (amortizes the ~352-cycle ACT instruction overhead).
  - Causal trimming on diagonal tiles: scores/exp/PV restricted to
    queries >= 128*d; the in-block triangle is zeroed on pt with a gpsimd
    affine_select (no DVE mask adds, no pt memsets).
  - exp on ScalarE straight out of PSUM with the 1/8 scale folded in; no
    max-subtraction (|scores|/8 stays well inside fp32 exp range).
  - Emission interleaves attention phases of chunk k-1 with the projection
    chains of chunk k so the PE instruction stream always has matmuls
    between exp-dependent PV steps (keeps the PE HAM clock-gate warm).
  - Weight/freq DMAs spread over the scalar/vector/gpsimd queues, x chunks
    and output tiles on the sync queue.
"""

import os
import sys

sys.path.insert(0, "/opt/trn_rl_repo")

import numpy as np
import ml_dtypes

BF16 = ml_dtypes.bfloat16

B, S, D = 2, 2048, 2048
NH = 4              # q heads per core
HD = 64             # head dim
KD = D // 128       # 16 contraction tiles
TT = S // 128       # 16 token tiles per batch
NCH = S // 512      # 4 query chunks per batch
SCALE = 1.0 / 8.0


def _build_bass():
    import functools

    import concourse.bacc as bacc
    import concourse.mybir as mybir
    from concourse.tile import TileContext
    from concourse.masks import make_identity

    # This kernel uses Exp (softmax) and Ln (1/d = exp(-ln d)) on ScalarE.
    # The act-table pass maps Exp -> "exp_and_others" and Ln ->
    # "natural_log", thrashing the 2.7us table load between them.  Narrow
    # the candidate sets (set names/indices preserved) so both resolve to
    # "natural_log_exp_and_others", which contains exp, ln AND copy ->
    # exactly one table load for the whole kernel.  Patch is scoped to this
    # build and restored afterwards.
    _orig_gat = bacc.get_activation_tables
    Exp_f = mybir.ActivationFunctionType.Exp
    Ln_f = mybir.ActivationFunctionType.Ln

    @functools.wraps(_orig_gat)
    def _gat(arch):
        tables = dict(_orig_gat(arch))
        out = {}
        for name, fns in tables.items():
            if name != "natural_log_exp_and_others":
                fns = fns - {Exp_f, Ln_f}
            out[name] = fns
        return out

    bacc.get_activation_tables = _gat

    f32 = mybir.dt.float32
    bf16 = mybir.dt.bfloat16
    Exp = mybir.ActivationFunctionType.Exp
    Copy = mybir.ActivationFunctionType.Copy
    Log = mybir.ActivationFunctionType.Ln

    nc = bacc.Bacc(None, target_bir_lowering=False)
    xT_d = nc.dram_tensor("xT", [B, D, S], bf16, kind="ExternalInput")
    wqkv_d = nc.dram_tensor("wqkv", [D, 384], bf16, kind="ExternalInput")
    wo01_d = nc.dram_tensor("wo01", [128, D], bf16, kind="ExternalInput")
    wo23_d = nc.dram_tensor("wo23", [128, D], bf16, kind="ExternalInput")
    cq_d = nc.dram_tensor("cq", [128, S], bf16, kind="ExternalInput")
    sq_d = nc.dram_tensor("sq", [128, S], bf16, kind="ExternalInput")
    out_d = nc.dram_tensor("out", [B, S, D], bf16, kind="ExternalOutput")

    with TileContext(nc) as tc:
        with (
            tc.tile_pool(name="const", bufs=1) as constp,
            tc.tile_pool(name="wts", bufs=1) as wtsp,
            tc.tile_pool(name="xin", bufs=2) as xinp,
            tc.tile_pool(name="kv8", bufs=10) as kv8p,
            tc.tile_pool(name="qch", bufs=6) as qchp,
            tc.tile_pool(name="och", bufs=6) as ochp,
            tc.tile_pool(name="qw", bufs=4) as qwp,
            tc.tile_pool(name="sh", bufs=3) as shp,
            tc.tile_pool(name="rt", bufs=4) as rtp,
            tc.tile_pool(name="pt", bufs=20) as ptp,
            tc.tile_pool(name="ri", bufs=6) as rip,
            tc.tile_pool(name="ost", bufs=3) as ostp,
            tc.tile_pool(name="pj", bufs=2, space="PSUM") as pjp,
            tc.tile_pool(name="sc", bufs=2, space="PSUM") as scp,
            tc.tile_pool(name="pv", bufs=2, space="PSUM") as pvp,
        ):
            # ---- constants ----
            ident = constp.tile([128, 128], bf16, name="ident")
            make_identity(nc, ident[:, :])

            # ---- weights / freqs spread across idle DMA queues so x chunk
            # 0 (sync queue) and wqkv (scalar queue) load in parallel ----
            # wqkv in two halves so the first projection matmuls can start
            # after ~half the weight bytes have landed
            wqkv_sbs = []
            for h in range(2):
                wq_h = wtsp.tile(
                    [128, (KD // 2) * 384], bf16, name=f"wqkv_sb{h}"
                )
                nc.scalar.dma_start(
                    out=wq_h[:, :].rearrange("p (k e) -> p k e", k=KD // 2),
                    in_=wqkv_d.rearrange("(k p) e -> p k e", p=128)[
                        :, h * (KD // 2) : (h + 1) * (KD // 2)
                    ],
                )
                wqkv_sbs.append(wq_h)
            cq_sb = wtsp.tile([128, S], bf16, name="cq_sb")
            nc.scalar.dma_start(out=cq_sb[:, :], in_=cq_d[:, :])
            sq_sb = wtsp.tile([128, S], bf16, name="sq_sb")
            nc.scalar.dma_start(out=sq_sb[:, :], in_=sq_d[:, :])

            # per-(batch,chunk) tile registries
            Xc = {}    # (b,c) -> [128, KD*512] x chunk (e-major)
            KRc = {}   # (b,c) -> [128,512] rotated K duplicated both halves
            Vc = {}    # (b,c) -> [128, 4*128] token-major [vT | ones] blocks
            QRc = {}   # (b,c) -> (QR01, QR23)
            OTc = {}   # (b,c) -> (oT01, oT23) normalized outputs

            def ensure_x(b, c):
                if (b, c) in Xc:
                    return
                csl = slice(c * 512, c * 512 + 512)
                halves = []
                for h in range(2):
                    xh = xinp.tile(
                        [128, (KD // 2) * 512], bf16, tag=f"xc{h}",
                        name=f"x{h}",
                    )
                    nc.sync.dma_start(
                        out=xh[:, :].rearrange("p (k t) -> p k t", k=KD // 2),
                        in_=xT_d[b, :, csl].rearrange(
                            "(k p) t -> p k t", p=128
                        )[:, h * (KD // 2) : (h + 1) * (KD // 2)],
                    )
                    halves.append(xh)
                Xc[(b, c)] = halves

            def emit_proj_chain(b, c, which):
                """One of three projection chains for chunk (b,c):
                which=0: q01 proj + rope; which=1: q23 proj + rope;
                which=2: kv proj + K rope + V transpose."""
                ensure_x(b, c)
                csl = slice(c * 512, c * 512 + 512)
                xh = Xc[(b, c)]
                et = which
                ps = pjp.tile([128, 512], f32, tag="pj", name="ps_prj")
                for kd in range(KD):
                    h, kh = divmod(kd, KD // 2)
                    nc.tensor.matmul(
                        ps[:, :],
                        lhsT=wqkv_sbs[h][
                            :, kh * 384 + et * 128 : kh * 384 + et * 128 + 128
                        ],
                        rhs=xh[h][:, kh * 512 : kh * 512 + 512],
                        start=(kd == 0),
                        stop=(kd == KD - 1),
                    )
                raw = qwp.tile([128, 512], bf16, tag="qraw", name="raw")
                if which == 0:
                    nc.scalar.activation(raw[:, :], ps[:, :], Copy)
                else:
                    nc.vector.tensor_copy(raw[:, :], ps[:, :])

                if which < 2:
                    # rope for a q pair
                    QR = qchp.tile([128, 512], bf16, tag="qrc", name="QR")
                    qsh = shp.tile([128, 512], bf16, tag="sh", name="qsh")
                    for blk in range(4):
                        src = (blk ^ 1) * 32
                        nc.vector.tensor_copy(
                            qsh[blk * 32 : blk * 32 + 32, :],
                            raw[src : src + 32, :],
                        )
                    t1 = rtp.tile([128, 512], bf16, tag="rt", name="t1")
                    t2 = rtp.tile([128, 512], bf16, tag="rt", name="t2")
                    nc.vector.tensor_mul(t1[:, :], raw[:, :], cq_sb[:, csl])
                    nc.vector.tensor_mul(t2[:, :], qsh[:, :], sq_sb[:, csl])
                    nc.vector.tensor_add(QR[:, :], t1[:, :], t2[:, :])
                    QRc.setdefault((b, c), {})[which] = QR
                    return

                # which == 2: V -> token-major [vT | ones] blocks
                v_sb = kv8p.tile([128, 4 * 128], bf16, tag="vc", name="v_sb")
                nc.gpsimd.memset(v_sb[:, :], 1.0)
                for ts in range(4):
                    ps_t = pjp.tile([128, 64], bf16, tag="pj", name="ps_t")
                    nc.tensor.transpose(
                        ps_t[:, 0:64],
                        raw[0:64, ts * 128 : ts * 128 + 128],
                        ident[0:64, 0:64],
                    )
                    nc.vector.tensor_copy(
                        v_sb[:, ts * 128 : ts * 128 + 64], ps_t[:, 0:64]
                    )
                Vc[(b, c)] = v_sb

                # rope for k (rows 64:128 of raw), duplicated into both
                # partition halves of KR
                KR = kv8p.tile([128, 512], bf16, tag="krc", name="KR")
                ksh = shp.tile([128, 512], bf16, tag="sh", name="ksh")
                nc.vector.tensor_copy(ksh[64:96, :], raw[96:128, :])
                nc.vector.tensor_copy(ksh[96:128, :], raw[64:96, :])
                k1 = rtp.tile([128, 512], bf16, tag="rt", name="k1")
                k2 = rtp.tile([128, 512], bf16, tag="rt", name="k2")
                nc.vector.tensor_mul(
                    k1[0:64, :], raw[64:128, :], cq_sb[64:128, csl]
                )
                nc.vector.tensor_mul(
                    k2[0:64, :], ksh[64:128, :], sq_sb[64:128, csl]
                )
                nc.vector.tensor_add(KR[0:64, :], k1[0:64, :], k2[0:64, :])
                nc.vector.tensor_add(KR[64:128, :], k1[0:64, :], k2[0:64, :])
                KRc[(b, c)] = KR

            def emit_scores(b, j, hp, pts):
                """scores + batched exp for query chunk j, head pair hp."""
                nts = 4 * j + 4
                QR = QRc[(b, j)][hp]
                for i in range(nts):
                    dd = i - 4 * j
                    trim = 128 * max(dd, 0)
                    KR = KRc[(b, i // 4)]
                    ksl = slice((i % 4) * 128, (i % 4) * 128 + 128)
                    ps_s = scp.tile([128, 1024], f32, tag="sc", name="ps_s")
                    for sub in range(2):
                        r0 = sub * 64
                        nc.tensor.matmul(
                            ps_s[:, sub * 512 + trim : sub * 512 + 512],
                            lhsT=KR[r0 : r0 + 64, ksl],
                            rhs=QR[r0 : r0 + 64, trim:512],
                            start=True,
                            stop=True,
                        )
                    pt = ptp.tile([128, 1024], bf16, tag="pt", name="pt")
                    if trim == 0:
                        nc.scalar.activation(
                            pt[:, :], ps_s[:, :], Exp, scale=SCALE
                        )
                    else:
                        ps3 = ps_s[:, :].rearrange("p (h t) -> p h t", h=2)
                        pt3 = pt[:, :].rearrange("p (h t) -> p h t", h=2)
                        nc.scalar.activation(
                            pt3[:, :, trim:512],
                            ps3[:, :, trim:512],
                            Exp,
                            scale=SCALE,
                        )
                    if dd >= 0:
                        if trim > 0:
                            # zero the fully-masked query range skipped by
                            # the trimmed exp (PV reads the full tile)
                            pt3z = pt[:, :].rearrange(
                                "p (h t) -> p h t", h=2
                            )
                            nc.gpsimd.memset(pt3z[:, :, 0:trim], 0.0)
                        # zero the in-block causal triangle (q < k)
                        for h2 in range(2):
                            blk = slice(
                                h2 * 512 + trim, h2 * 512 + trim + 128
                            )
                            nc.gpsimd.affine_select(
                                out=pt[:, blk],
                                in_=pt[:, blk],
                                compare_op=mybir.AluOpType.is_ge,
                                fill=0.0,
                                base=0,
                                pattern=[[1, 128]],
                                channel_multiplier=-1,
                            )
                    pts[(i, hp)] = pt

            def emit_pv(b, j, hp, pts):
                """PV chains + fused normalize for head pair hp of chunk j."""
                nts = 4 * j + 4
                oT = ochp.tile([128, 512], bf16, tag="otc", name="oT")
                for sub in range(2):
                    r0 = sub * 64
                    ps_pv = pvp.tile([128, 512], f32, tag="pv", name="ps_pv")
                    for i in range(nts):
                        v_sb = Vc[(b, i // 4)]
                        vsl = slice((i % 4) * 128, (i % 4) * 128 + 128)
                        nc.tensor.matmul(
                            ps_pv[:, :],
                            lhsT=v_sb[:, vsl],
                            rhs=pts[(i, hp)][:, sub * 512 : sub * 512 + 512],
                            start=(i == 0),
                            stop=(i == nts - 1),
                        )
                    # 1/d = exp(-ln(d)) on ScalarE: Log and Exp share the
                    # natural_log_exp_and_others table set (no switch cost)
                    lnd = rip.tile([64, 512], f32, tag="ln", name="lnd")
                    rinv = rip.tile([64, 512], f32, tag="ri", name="rinv")
                    nc.scalar.activation(
                        lnd[:, :], ps_pv[64:128, :], Log
                    )
                    nc.scalar.activation(
                        rinv[:, :], lnd[:, :], Exp, scale=-1.0
                    )
                    nc.vector.tensor_mul(
                        oT[r0 : r0 + 64, :], ps_pv[0:64, :], rinv[:, :]
                    )
                OTc.setdefault((b, j), {})[hp] = oT

            def emit_outproj(b, j):
                """output projection for the 4 token tiles of chunk j."""
                oT01 = OTc[(b, j)][0]
                oT23 = OTc[(b, j)][1]
                for ts in range(4):
                    tt = j * 4 + ts
                    tsl = slice(ts * 128, ts * 128 + 128)
                    ot = ostp.tile([128, D], bf16, tag="ot", name="ot")
                    for dmc in range(4):
                        po = pjp.tile([128, 512], f32, tag="pj", name="po")
                        nc.tensor.matmul(
                            po[:, :],
                            lhsT=oT01[:, tsl],
                            rhs=wo01_sb[:, dmc * 512 : dmc * 512 + 512],
                            start=True,
                            stop=False,
                        )
                        nc.tensor.matmul(
                            po[:, :],
                            lhsT=oT23[:, tsl],
                            rhs=wo23_sb[:, dmc * 512 : dmc * 512 + 512],
                            start=False,
                            stop=True,
                        )
                        if dmc == 0:
                            nc.scalar.activation(
                                ot[:, dmc * 512 : dmc * 512 + 512],
                                po[:, :],
                                Copy,
                            )
                        else:
                            nc.vector.tensor_copy(
                                ot[:, dmc * 512 : dmc * 512 + 512], po[:, :]
                            )
                    nc.sync.dma_start(
                        out=out_d[b, tt * 128 : tt * 128 + 128, :],
                        in_=ot[:, :],
                    )

            # emission: interleave attention phases of chunk k-1 with the
            # projection chains of chunk k so the PE stream always has
            # exp-independent matmuls to fill exp-wait gaps.
            chunks = [(b, c) for b in range(B) for c in range(NCH)]
            ensure_x(*chunks[0])
            # wo loads queue on sync AFTER the first x chunk (needed ~40us
            # in, while x0 gates the very first projection matmul)
            wo01_sb = wtsp.tile([128, D], bf16, name="wo01_sb")
            nc.sync.dma_start(out=wo01_sb[:, :], in_=wo01_d[:, :])
            wo23_sb = wtsp.tile([128, D], bf16, name="wo23_sb")
            nc.sync.dma_start(out=wo23_sb[:, :], in_=wo23_d[:, :])
            prev_pts = None
            for idx, (b, c) in enumerate(chunks):
                A = chunks[idx - 1] if idx >= 1 else None
                pts = {}
                if A is not None:
                    emit_scores(A[0], A[1], 0, pts)
                emit_proj_chain(b, c, 0)
                if idx + 1 < len(chunks):
                    ensure_x(*chunks[idx + 1])
                if A is not None:
                    emit_pv(A[0], A[1], 0, pts)
                emit_proj_chain(b, c, 1)
                if A is not None:
                    emit_scores(A[0], A[1], 1, pts)
                emit_proj_chain(b, c, 2)
                if A is not None:
                    emit_pv(A[0], A[1], 1, pts)
                    emit_outproj(A[0], A[1])
            # tail: attention + outproj of the final chunk
            A = chunks[-1]
            pts = {}
            emit_scores(A[0], A[1], 0, pts)
            emit_pv(A[0], A[1], 0, pts)
            emit_scores(A[0], A[1], 1, pts)
            emit_pv(A[0], A[1], 1, pts)
            emit_outproj(A[0], A[1])
    try:
        nc.compile()
    finally:
        bacc.get_activation_tables = _orig_gat
    return nc


def _prep_in_maps(inputs):
    x = np.asarray(inputs["x"], dtype=np.float32)
    fc = np.asarray(inputs["freqs_cos"], dtype=np.float32)
    fs = np.asarray(inputs["freqs_sin"], dtype=np.float32)
    wq = np.asarray(inputs["wq"], dtype=np.float32)
    wk = np.asarray(inputs["wk"], dtype=np.float32)
    wv = np.asarray(inputs["wv"], dtype=np.float32)
    wo = np.asarray(inputs["wo"], dtype=np.float32)

    xT = np.ascontiguousarray(np.transpose(x, (0, 2, 1))).astype(BF16)
    c = np.ascontiguousarray(fc.T)  # [32, S]
    s = np.ascontiguousarray(fs.T)
    cq = np.concatenate([c, c, c, c], axis=0).astype(BF16)      # [128, S]
    sq = np.concatenate([-s, s, -s, s], axis=0).astype(BF16)    # [128, S]
    perm = np.concatenate([np.arange(0, 64, 2), np.arange(1, 64, 2)])

    in_maps = []
    for m in range(8):
        wqs = wq[:, m * 256 : m * 256 + 256].reshape(D, 4, 64)[:, :, perm]
        wq01 = wqs[:, 0:2].reshape(D, 128)
        wq23 = wqs[:, 2:4].reshape(D, 128)
        wks = wk[:, m * 64 : m * 64 + 64][:, perm]
        wvs = wv[:, m * 64 : m * 64 + 64]
        wkv = np.concatenate([wvs, wks], axis=1)  # vT rows 0:64, k rows 64:128
        wqkv = np.ascontiguousarray(
            np.concatenate([wq01, wq23, wkv], axis=1)
        ).astype(BF16)
        wo01 = np.ascontiguousarray(wo[m * 256 : m * 256 + 128, :]).astype(BF16)
        wo23 = np.ascontiguousarray(wo[m * 256 + 128 : m * 256 + 256, :]).astype(
            BF16
        )
        in_maps.append(
            dict(xT=xT, wqkv=wqkv, wo01=wo01, wo23=wo23, cq=cq, sq=sq)
        )
    return in_maps


LAST_EXEC_NS = None


def _install_ntff_hook():
    """Provide antenv.axon_hooks (missing in some containers) so that
    run_bass_kernel_spmd(trace=True) can capture an NTFF profile."""
    import types

    try:
        import antenv.axon_hooks  # noqa: F401
        return True
    except ImportError:
        pass
    try:
        import antenv
        from trn_agent_boot.trn_boot import _ntff_profile_via_ctypes

        hook = _ntff_profile_via_ctypes("/opt/axon/libaxon_pjrt.so")
        if hook is None:
            return False
        mod = types.ModuleType("antenv.axon_hooks")
        mod._hook = hook
        mod.set_axon_ntff_profile_hook = lambda h: setattr(mod, "_hook", h)
        mod.get_axon_ntff_profile_hook = lambda: mod._hook
        sys.modules["antenv.axon_hooks"] = mod
        antenv.axon_hooks = mod
        return True
    except Exception:
        return False


def kernel(**inputs):
    global LAST_EXEC_NS
    from concourse import bass_utils

    in_maps = _prep_in_maps(inputs)
    nc = _build_bass()
    trace = bool(int(os.environ.get("KERNEL_TRACE", "0")))
    if trace:
        trace = _install_ntff_hook()
    res = bass_utils.run_bass_kernel_spmd(
        nc, in_maps, core_ids=list(range(8)), trace=trace
    )
    if trace and res.exec_time_ns is not None:
        LAST_EXEC_NS = res.exec_time_ns
    out = np.zeros((B, S, D), dtype=np.float32)
    for r in res.results:
        out += r["out"].astype(np.float32)
    return out


def time_device(reps=6, **inputs):
    """Wall-clock the sharded PJRT executable with device-resident inputs
    (fallback when NTFF profiling is unavailable; includes axon dispatch
    overhead)."""
    import jax
    from concourse import bass2jax
    import concourse.mybir as mybir
    import time as _time

    in_maps = _prep_in_maps(inputs)
    nc = _build_bass()
    bass2jax.install_neuronx_cc_hook()

    partition_name = (
        nc.partition_id_tensor.name if nc.partition_id_tensor else None
    )
    in_names, out_names, out_avals, zero_outs = [], [], [], []
    for alloc in nc.m.functions[0].allocations:
        if not isinstance(alloc, mybir.MemoryLocationSet):
            continue
        name = alloc.memorylocations[0].name
        if alloc.kind == "ExternalInput":
            if name != partition_name:
                in_names.append(name)
        elif alloc.kind == "ExternalOutput":
            out_names.append(name)
            shape = tuple(alloc.tensor_shape)
            dt = mybir.dt.np(alloc.dtype)
            out_avals.append(jax.core.ShapedArray(shape, dt))
            zero_outs.append(np.zeros(shape, dt))
    n_params = len(in_names)
    in_all = in_names + out_names
    if partition_name is not None:
        in_all = in_all + [partition_name]

    def _body(*args):
        operands = list(args)
        if partition_name is not None:
            operands.append(bass2jax.partition_id_tensor())
        outs = bass2jax._bass_exec_p.bind(
            *operands,
            out_avals=tuple(out_avals),
            in_names=tuple(in_all),
            out_names=tuple(out_names),
            lowering_input_output_aliases=(),
            sim_require_finite=True,
            sim_require_nnan=True,
            nc=nc,
        )
        return tuple(outs)

    devices = jax.devices()[:8]
    mesh = bass2jax.Mesh(np.asarray(devices), ("core",))
    spec = bass2jax.PartitionSpec("core")
    nin = n_params + len(out_names)
    f = jax.jit(
        bass2jax.shard_map(
            _body,
            mesh=mesh,
            in_specs=(spec,) * nin,
            out_specs=(spec,) * len(out_names),
            check_rep=False,
        )
    )
    concat_in = [
        np.concatenate([np.asarray(m[n]) for m in in_maps], axis=0)
        for n in in_names
    ]
    concat_zeros = [
        np.zeros((8 * z.shape[0], *z.shape[1:]), z.dtype) for z in zero_outs
    ]
    sharding = jax.sharding.NamedSharding(mesh, spec)
    dev_args = [jax.device_put(a, sharding) for a in concat_in + concat_zeros]
    r = f(*dev_args)
    jax.block_until_ready(r)
    best = None
    for _ in range(reps):
        t0 = _time.perf_counter()
        r = f(*dev_args)
        jax.block_until_ready(r)
        dt = _time.perf_counter() - t0
        best = dt if best is None else min(best, dt)
    return int(best * 1e9)


# revision 31
# speedup vs baseline: 1.0239x; 1.0239x over previous
"""Trainium2 Bass kernel for GQA attention (nn_Attention_34832184770944).

Sharding: tensor-parallel across heads on 8 cores. Core m gets KV head m and
Q heads 4m..4m+3: wq/wk/wv sharded column-wise, wo row-wise. Each core
computes a full-shape bf16 partial output; the host sums the 8 partials in
fp32.

Device kernel design (v2, restructured from the 478us baseline):
  - QKV projection computed e-major (lhsT = weight tile, rhs = xT chunk of
    512 tokens) so Q and K come out pre-transposed; no PE transpose pass.
  - Host permutes wq/wk columns within each head (even indices first) so
    RoPE becomes NeoX rotate-half: partition-shift DVE copies + full-width
    bf16 tensor_tensor ops per chunk.
  - V is projected e-major then PE-transposed to token-major; each 128-col
    V block is [vT (64) | ones (64)], so the PV matmul emits softmax
    denominators REPLICATED on PSUM partitions 64:128 at zero extra cost
    (matmul time is set by rhs streaming, not output rows).
  - Normalize fused out of PSUM: reciprocal_approx_fast on rows 64:128 then
    one tensor_mul (PSUM rows 0:64 x rinv -> oT bf16). No DRAM round trip,
    no 1-partition reciprocal, no broadcast matmul, no separate oU copy.
  - Scores per (key tile, head pair): two concurrent row-group matmuls
    (heads at PE row strips 0/64 via KR duplicated into both halves) write
    one [128,1024] fp32 2-bank PSUM tile; ONE batched exp per pair tile
    (amortizes the ~352-cycle ACT instruction overhead).
  - Causal trimming on diagonal tiles: scores/exp/PV restricted to
    queries >= 128*d; the in-block triangle is zeroed on pt with a gpsimd
    affine_select (no DVE mask adds, no pt memsets).
  - exp on ScalarE straight out of PSUM with the 1/8 scale folded in; no
    max-subtraction (|scores|/8 stays well inside fp32 exp range).
  - Emission interleaves attention phases of chunk k-1 with the projection
    chains of chunk k so the PE instruction stream always has matmuls
    between exp-dependent PV steps (keeps the PE HAM clock-gate warm).
  - Weight/freq DMAs spread over the scalar/vector/gpsimd queues, x chunks
    and output tiles on the sync queue.
"""

import os
import sys

sys.path.insert(0, "/opt/trn_rl_repo")

import numpy as np
import ml_dtypes

BF16 = ml_dtypes.bfloat16

B, S, D = 2, 2048, 2048
NH = 4              # q heads per core
HD = 64             # head dim
KD = D // 128       # 16 contraction tiles
TT = S // 128       # 16 token tiles per batch
NCH = S // 512      # 4 query chunks per batch
SCALE = 1.0 / 8.0


def _build_bass():
    import functools

    import concourse.bacc as bacc
    import concourse.mybir as mybir
    from concourse.tile import TileContext
    from concourse.masks import make_identity

    # This kernel uses Exp (softmax) and Ln (1/d = exp(-ln d)) on ScalarE.
    # The act-table pass maps Exp -> "exp_and_others" and Ln ->
    # "natural_log", thrashing the 2.7us table load between them.  Narrow
    # the candidate sets (set names/indices preserved) so both resolve to
    # "natural_log_exp_and_others", which contains exp, ln AND copy ->
    # exactly one table load for the whole kernel.  Patch is scoped to this
    # build and restored afterwards.
    _orig_gat = bacc.get_activation_tables
    Exp_f = mybir.ActivationFunctionType.Exp
    Ln_f = mybir.ActivationFunctionType.Ln

    @functools.wraps(_orig_gat)
    def _gat(arch):
        tables = dict(_orig_gat(arch))
        out = {}
        for name, fns in tables.items():
            if name != "natural_log_exp_and_others":
                fns = fns - {Exp_f, Ln_f}
            out[name] = fns
        return out

    bacc.get_activation_tables = _gat

    f32 = mybir.dt.float32
    bf16 = mybir.dt.bfloat16
    Exp = mybir.ActivationFunctionType.Exp
    Copy = mybir.ActivationFunctionType.Copy
    Log = mybir.ActivationFunctionType.Ln

    nc = bacc.Bacc(None, target_bir_lowering=False)
    xT_d = nc.dram_tensor("xT", [B, D, S], bf16, kind="ExternalInput")
    wqkv_d = nc.dram_tensor("wqkv", [D, 384], bf16, kind="ExternalInput")
    wo01_d = nc.dram_tensor("wo01", [128, D], bf16, kind="ExternalInput")
    wo23_d = nc.dram_tensor("wo23", [128, D], bf16, kind="ExternalInput")
    cq_d = nc.dram_tensor("cq", [128, S], bf16, kind="ExternalInput")
    sq_d = nc.dram_tensor("sq", [128, S], bf16, kind="ExternalInput")
    out_d = nc.dram_tensor("out", [B, S, D], bf16, kind="ExternalOutput")

    with TileContext(nc) as tc:
        with (
            tc.tile_pool(name="const", bufs=1) as constp,
            tc.tile_pool(name="wts", bufs=1) as wtsp,
            tc.tile_pool(name="xin", bufs=2) as xinp,
            tc.tile_pool(name="kv8", bufs=10) as kv8p,
            tc.tile_pool(name="qch", bufs=6) as qchp,
            tc.tile_pool(name="och", bufs=6) as ochp,
            tc.tile_pool(name="qw", bufs=4) as qwp,
            tc.tile_pool(name="sh", bufs=3) as shp,
            tc.tile_pool(name="rt", bufs=4) as rtp,
            tc.tile_pool(name="pt", bufs=20) as ptp,
            tc.tile_pool(name="ri", bufs=6) as rip,
            tc.tile_pool(name="ost", bufs=3) as ostp,
            tc.tile_pool(name="pj", bufs=2, space="PSUM") as pjp,
            tc.tile_pool(name="sc", bufs=2, space="PSUM") as scp,
            tc.tile_pool(name="pv", bufs=2, space="PSUM") as pvp,
        ):
            # ---- constants ----
            ident = constp.tile([128, 128], bf16, name="ident")
            make_identity(nc, ident[:, :])

            # ---- weights / freqs spread across idle DMA queues so x chunk
            # 0 (sync queue) and wqkv (scalar queue) load in parallel ----
            # wqkv in four quarters so the first projection matmuls can
            # start as soon as the first weight slice lands
            NSP = 4
            KQ = KD // NSP
            wqkv_sbs = []
            for h in range(NSP):
                wq_h = wtsp.tile([128, KQ * 384], bf16, name=f"wqkv_sb{h}")
                nc.scalar.dma_start(
                    out=wq_h[:, :].rearrange("p (k e) -> p k e", k=KQ),
                    in_=wqkv_d.rearrange("(k p) e -> p k e", p=128)[
                        :, h * KQ : (h + 1) * KQ
                    ],
                )
                wqkv_sbs.append(wq_h)
            cq_sb = wtsp.tile([128, S], bf16, name="cq_sb")
            nc.scalar.dma_start(out=cq_sb[:, :], in_=cq_d[:, :])
            sq_sb = wtsp.tile([128, S], bf16, name="sq_sb")
            nc.scalar.dma_start(out=sq_sb[:, :], in_=sq_d[:, :])

            # per-(batch,chunk) tile registries
            Xc = {}    # (b,c) -> [128, KD*512] x chunk (e-major)
            KRc = {}   # (b,c) -> [128,512] rotated K duplicated both halves
            Vc = {}    # (b,c) -> [128, 4*128] token-major [vT | ones] blocks
            QRc = {}   # (b,c) -> (QR01, QR23)
            OTc = {}   # (b,c) -> (oT01, oT23) normalized outputs

            def ensure_x(b, c):
                if (b, c) in Xc:
                    return
                csl = slice(c * 512, c * 512 + 512)
                halves = []
                for h in range(NSP):
                    xh = xinp.tile(
                        [128, KQ * 512], bf16, tag=f"xc{h}", name=f"x{h}"
                    )
                    nc.sync.dma_start(
                        out=xh[:, :].rearrange("p (k t) -> p k t", k=KQ),
                        in_=xT_d[b, :, csl].rearrange(
                            "(k p) t -> p k t", p=128
                        )[:, h * KQ : (h + 1) * KQ],
                    )
                    halves.append(xh)
                Xc[(b, c)] = halves

            def emit_proj_chain(b, c, which):
                """One of three projection chains for chunk (b,c):
                which=0: q01 proj + rope; which=1: q23 proj + rope;
                which=2: kv proj + K rope + V transpose."""
                ensure_x(b, c)
                csl = slice(c * 512, c * 512 + 512)
                xh = Xc[(b, c)]
                et = which
                ps = pjp.tile([128, 512], f32, tag="pj", name="ps_prj")
                for kd in range(KD):
                    h, kh = divmod(kd, KQ)
                    nc.tensor.matmul(
                        ps[:, :],
                        lhsT=wqkv_sbs[h][
                            :, kh * 384 + et * 128 : kh * 384 + et * 128 + 128
                        ],
                        rhs=xh[h][:, kh * 512 : kh * 512 + 512],
                        start=(kd == 0),
                        stop=(kd == KD - 1),
                    )
                raw = qwp.tile([128, 512], bf16, tag="qraw", name="raw")
                if which == 0:
                    nc.scalar.activation(raw[:, :], ps[:, :], Copy)
                else:
                    nc.vector.tensor_copy(raw[:, :], ps[:, :])

                if which < 2:
                    # rope for a q pair
                    QR = qchp.tile([128, 512], bf16, tag="qrc", name="QR")
                    qsh = shp.tile([128, 512], bf16, tag="sh", name="qsh")
                    for blk in range(4):
                        src = (blk ^ 1) * 32
                        nc.vector.tensor_copy(
                            qsh[blk * 32 : blk * 32 + 32, :],
                            raw[src : src + 32, :],
                        )
                    t1 = rtp.tile([128, 512], bf16, tag="rt", name="t1")
                    t2 = rtp.tile([128, 512], bf16, tag="rt", name="t2")
                    nc.vector.tensor_mul(t1[:, :], raw[:, :], cq_sb[:, csl])
                    nc.vector.tensor_mul(t2[:, :], qsh[:, :], sq_sb[:, csl])
                    nc.vector.tensor_add(QR[:, :], t1[:, :], t2[:, :])
                    QRc.setdefault((b, c), {})[which] = QR
                    return

                # which == 2: V -> token-major [vT | ones] blocks
                v_sb = kv8p.tile([128, 4 * 128], bf16, tag="vc", name="v_sb")
                nc.gpsimd.memset(v_sb[:, :], 1.0)
                for ts in range(4):
                    ps_t = pjp.tile([128, 64], bf16, tag="pj", name="ps_t")
                    nc.tensor.transpose(
                        ps_t[:, 0:64],
                        raw[0:64, ts * 128 : ts * 128 + 128],
                        ident[0:64, 0:64],
                    )
                    nc.vector.tensor_copy(
                        v_sb[:, ts * 128 : ts * 128 + 64], ps_t[:, 0:64]
                    )
                Vc[(b, c)] = v_sb

                # rope for k (rows 64:128 of raw), duplicated into both
                # partition halves of KR
                KR = kv8p.tile([128, 512], bf16, tag="krc", name="KR")
                ksh = shp.tile([128, 512], bf16, tag="sh", name="ksh")
                nc.vector.tensor_copy(ksh[64:96, :], raw[96:128, :])
                nc.vector.tensor_copy(ksh[96:128, :], raw[64:96, :])
                k1 = rtp.tile([128, 512], bf16, tag="rt", name="k1")
                k2 = rtp.tile([128, 512], bf16, tag="rt", name="k2")
                nc.vector.tensor_mul(
                    k1[0:64, :], raw[64:128, :], cq_sb[64:128, csl]
                )
                nc.vector.tensor_mul(
                    k2[0:64, :], ksh[64:128, :], sq_sb[64:128, csl]
                )
                nc.vector.tensor_add(KR[0:64, :], k1[0:64, :], k2[0:64, :])
                nc.vector.tensor_add(KR[64:128, :], k1[0:64, :], k2[0:64, :])
                KRc[(b, c)] = KR

            def emit_scores(b, j, hp, pts):
                """scores + batched exp for query chunk j, head pair hp."""
                nts = 4 * j + 4
                QR = QRc[(b, j)][hp]
                for i in range(nts):
                    dd = i - 4 * j
                    trim = 128 * max(dd, 0)
                    KR = KRc[(b, i // 4)]
                    ksl = slice((i % 4) * 128, (i % 4) * 128 + 128)
                    ps_s = scp.tile([128, 1024], f32, tag="sc", name="ps_s")
                    for sub in range(2):
                        r0 = sub * 64
                        nc.tensor.matmul(
                            ps_s[:, sub * 512 + trim : sub * 512 + 512],
                            lhsT=KR[r0 : r0 + 64, ksl],
                            rhs=QR[r0 : r0 + 64, trim:512],
                            start=True,
                            stop=True,
                        )
                    pt = ptp.tile([128, 1024], bf16, tag="pt", name="pt")
                    if trim == 0:
                        nc.scalar.activation(
                            pt[:, :], ps_s[:, :], Exp, scale=SCALE
                        )
                    else:
                        ps3 = ps_s[:, :].rearrange("p (h t) -> p h t", h=2)
                        pt3 = pt[:, :].rearrange("p (h t) -> p h t", h=2)
                        nc.scalar.activation(
                            pt3[:, :, trim:512],
                            ps3[:, :, trim:512],
                            Exp,
                            scale=SCALE,
                        )
                    if dd >= 0:
                        if trim > 0:
                            # zero the fully-masked query range skipped by
                            # the trimmed exp (PV reads the full tile)
                            pt3z = pt[:, :].rearrange(
                                "p (h t) -> p h t", h=2
                            )
                            nc.gpsimd.memset(pt3z[:, :, 0:trim], 0.0)
                        # zero the in-block causal triangle (q < k)
                        for h2 in range(2):
                            blk = slice(
                                h2 * 512 + trim, h2 * 512 + trim + 128
                            )
                            nc.gpsimd.affine_select(
                                out=pt[:, blk],
                                in_=pt[:, blk],
                                compare_op=mybir.AluOpType.is_ge,
                                fill=0.0,
                                base=0,
                                pattern=[[1, 128]],
                                channel_multiplier=-1,
                            )
                    pts[(i, hp)] = pt

            def emit_pv(b, j, hp, pts):
                """PV chains + fused normalize for head pair hp of chunk j."""
                nts = 4 * j + 4
                oT = ochp.tile([128, 512], bf16, tag="otc", name="oT")
                for sub in range(2):
                    r0 = sub * 64
                    ps_pv = pvp.tile([128, 512], f32, tag="pv", name="ps_pv")
                    for i in range(nts):
                        v_sb = Vc[(b, i // 4)]
                        vsl = slice((i % 4) * 128, (i % 4) * 128 + 128)
                        nc.tensor.matmul(
                            ps_pv[:, :],
                            lhsT=v_sb[:, vsl],
                            rhs=pts[(i, hp)][:, sub * 512 : sub * 512 + 512],
                            start=(i == 0),
                            stop=(i == nts - 1),
                        )
                    # 1/d = exp(-ln(d)) on ScalarE: Log and Exp share the
                    # natural_log_exp_and_others table set (no switch cost)
                    lnd = rip.tile([64, 512], f32, tag="ln", name="lnd")
                    rinv = rip.tile([64, 512], f32, tag="ri", name="rinv")
                    nc.scalar.activation(
                        lnd[:, :], ps_pv[64:128, :], Log
                    )
                    nc.scalar.activation(
                        rinv[:, :], lnd[:, :], Exp, scale=-1.0
                    )
                    nc.vector.tensor_mul(
                        oT[r0 : r0 + 64, :], ps_pv[0:64, :], rinv[:, :]
                    )
                OTc.setdefault((b, j), {})[hp] = oT

            def emit_outproj(b, j):
                """output projection for the 4 token tiles of chunk j."""
                oT01 = OTc[(b, j)][0]
                oT23 = OTc[(b, j)][1]
                for ts in range(4):
                    tt = j * 4 + ts
                    tsl = slice(ts * 128, ts * 128 + 128)
                    ot = ostp.tile([128, D], bf16, tag="ot", name="ot")
                    for dmc in range(4):
                        po = pjp.tile([128, 512], f32, tag="pj", name="po")
                        nc.tensor.matmul(
                            po[:, :],
                            lhsT=oT01[:, tsl],
                            rhs=wo01_sb[:, dmc * 512 : dmc * 512 + 512],
                            start=True,
                            stop=False,
                        )
                        nc.tensor.matmul(
                            po[:, :],
                            lhsT=oT23[:, tsl],
                            rhs=wo23_sb[:, dmc * 512 : dmc * 512 + 512],
                            start=False,
                            stop=True,
                        )
                        if dmc == 0:
                            nc.scalar.activation(
                                ot[:, dmc * 512 : dmc * 512 + 512],
                                po[:, :],
                                Copy,
                            )
                        else:
                            nc.vector.tensor_copy(
                                ot[:, dmc * 512 : dmc * 512 + 512], po[:, :]
                            )
                    nc.sync.dma_start(
                        out=out_d[b, tt * 128 : tt * 128 + 128, :],
                        in_=ot[:, :],
                    )

            # emission: interleave attention phases of chunk k-1 with the
            # projection chains of chunk k so the PE stream always has
            # exp-independent matmuls to fill exp-wait gaps.
            chunks = [(b, c) for b in range(B) for c in range(NCH)]
            ensure_x(*chunks[0])
            # wo loads queue on sync AFTER the first x chunk (needed ~40us
            # in, while x0 gates the very first projection matmul)
            wo01_sb = wtsp.tile([128, D], bf16, name="wo01_sb")
            nc.sync.dma_start(out=wo01_sb[:, :], in_=wo01_d[:, :])
            wo23_sb = wtsp.tile([128, D], bf16, name="wo23_sb")
            nc.sync.dma_start(out=wo23_sb[:, :], in_=wo23_d[:, :])
            for idx, (b, c) in enumerate(chunks):
                A = chunks[idx - 1] if idx >= 1 else None
                pts = {}
                if A is not None:
                    emit_scores(A[0], A[1], 0, pts)
                emit_proj_chain(b, c, 0)
                if idx + 1 < len(chunks):
                    ensure_x(*chunks[idx + 1])
                if A is not None:
                    emit_pv(A[0], A[1], 0, pts)
                emit_proj_chain(b, c, 1)
                if A is not None:
                    emit_scores(A[0], A[1], 1, pts)
                emit_proj_chain(b, c, 2)
                if A is not None:
                    emit_pv(A[0], A[1], 1, pts)
                    emit_outproj(A[0], A[1])
            # tail: attention + outproj of the final chunk
            A = chunks[-1]
            pts = {}
            emit_scores(A[0], A[1], 0, pts)
            emit_pv(A[0], A[1], 0, pts)
            emit_scores(A[0], A[1], 1, pts)
            emit_pv(A[0], A[1], 1, pts)
            emit_outproj(A[0], A[1])
    try:
        nc.compile()
    finally:
        bacc.get_activation_tables = _orig_gat
    return nc


def _prep_in_maps(inputs):
    x = np.asarray(inputs["x"], dtype=np.float32)
    fc = np.asarray(inputs["freqs_cos"], dtype=np.float32)
    fs = np.asarray(inputs["freqs_sin"], dtype=np.float32)
    wq = np.asarray(inputs["wq"], dtype=np.float32)
    wk = np.asarray(inputs["wk"], dtype=np.float32)
    wv = np.asarray(inputs["wv"], dtype=np.float32)
    wo = np.asarray(inputs["wo"], dtype=np.float32)

    xT = np.ascontiguousarray(np.transpose(x, (0, 2, 1))).astype(BF16)
    c = np.ascontiguousarray(fc.T)  # [32, S]
    s = np.ascontiguousarray(fs.T)
    cq = np.concatenate([c, c, c, c], axis=0).astype(BF16)      # [128, S]
    sq = np.concatenate([-s, s, -s, s], axis=0).astype(BF16)    # [128, S]
    perm = np.concatenate([np.arange(0, 64, 2), np.arange(1, 64, 2)])

    in_maps = []
    for m in range(8):
        wqs = wq[:, m * 256 : m * 256 + 256].reshape(D, 4, 64)[:, :, perm]
        wq01 = wqs[:, 0:2].reshape(D, 128)
        wq23 = wqs[:, 2:4].reshape(D, 128)
        wks = wk[:, m * 64 : m * 64 + 64][:, perm]
        wvs = wv[:, m * 64 : m * 64 + 64]
        wkv = np.concatenate([wvs, wks], axis=1)  # vT rows 0:64, k rows 64:128
        wqkv = np.ascontiguousarray(
            np.concatenate([wq01, wq23, wkv], axis=1)
        ).astype(BF16)
        wo01 = np.ascontiguousarray(wo[m * 256 : m * 256 + 128, :]).astype(BF16)
        wo23 = np.ascontiguousarray(wo[m * 256 + 128 : m * 256 + 256, :]).astype(
            BF16
        )
        in_maps.append(
            dict(xT=xT, wqkv=wqkv, wo01=wo01, wo23=wo23, cq=cq, sq=sq)
        )
    return in_maps


LAST_EXEC_NS = None


def _install_ntff_hook():
    """Provide antenv.axon_hooks (missing in some containers) so that
    run_bass_kernel_spmd(trace=True) can capture an NTFF profile."""
    import types

    try:
        import antenv.axon_hooks  # noqa: F401
        return True
    except ImportError:
        pass
    try:
        import antenv
        from trn_agent_boot.trn_boot import _ntff_profile_via_ctypes

        hook = _ntff_profile_via_ctypes("/opt/axon/libaxon_pjrt.so")
        if hook is None:
            return False
        mod = types.ModuleType("antenv.axon_hooks")
        mod._hook = hook
        mod.set_axon_ntff_profile_hook = lambda h: setattr(mod, "_hook", h)
        mod.get_axon_ntff_profile_hook = lambda: mod._hook
        sys.modules["antenv.axon_hooks"] = mod
        antenv.axon_hooks = mod
        return True
    except Exception:
        return False


def kernel(**inputs):
    global LAST_EXEC_NS
    from concourse import bass_utils

    in_maps = _prep_in_maps(inputs)
    nc = _build_bass()
    trace = bool(int(os.environ.get("KERNEL_TRACE", "0")))
    if trace:
        trace = _install_ntff_hook()
    res = bass_utils.run_bass_kernel_spmd(
        nc, in_maps, core_ids=list(range(8)), trace=trace
    )
    if trace and res.exec_time_ns is not None:
        LAST_EXEC_NS = res.exec_time_ns
    out = np.zeros((B, S, D), dtype=np.float32)
    for r in res.results:
        out += r["out"].astype(np.float32)
    return out


def time_device(reps=6, **inputs):
    """Wall-clock the sharded PJRT executable with device-resident inputs
    (fallback when NTFF profiling is unavailable; includes axon dispatch
    overhead)."""
    import jax
    from concourse import bass2jax
    import concourse.mybir as mybir
    import time as _time

    in_maps = _prep_in_maps(inputs)
    nc = _build_bass()
    bass2jax.install_neuronx_cc_hook()

    partition_name = (
        nc.partition_id_tensor.name if nc.partition_id_tensor else None
    )
    in_names, out_names, out_avals, zero_outs = [], [], [], []
    for alloc in nc.m.functions[0].allocations:
        if not isinstance(alloc, mybir.MemoryLocationSet):
            continue
        name = alloc.memorylocations[0].name
        if alloc.kind == "ExternalInput":
            if name != partition_name:
                in_names.append(name)
        elif alloc.kind == "ExternalOutput":
            out_names.append(name)
            shape = tuple(alloc.tensor_shape)
            dt = mybir.dt.np(alloc.dtype)
            out_avals.append(jax.core.ShapedArray(shape, dt))
            zero_outs.append(np.zeros(shape, dt))
    n_params = len(in_names)
    in_all = in_names + out_names
    if partition_name is not None:
        in_all = in_all + [partition_name]

    def _body(*args):
        operands = list(args)
        if partition_name is not None:
            operands.append(bass2jax.partition_id_tensor())
        outs = bass2jax._bass_exec_p.bind(
            *operands,
            out_avals=tuple(out_avals),
            in_names=tuple(in_all),
            out_names=tuple(out_names),
            lowering_input_output_aliases=(),
            sim_require_finite=True,
            sim_require_nnan=True,
            nc=nc,
        )
        return tuple(outs)

    devices = jax.devices()[:8]
    mesh = bass2jax.Mesh(np.asarray(devices), ("core",))
    spec = bass2jax.PartitionSpec("core")
    nin = n_params + len(out_names)
    f = jax.jit(
        bass2jax.shard_map(
            _body,
            mesh=mesh,
            in_specs=(spec,) * nin,
            out_specs=(spec,) * len(out_names),
            check_rep=False,
        )
    )
    concat_in = [
        np.concatenate([np.asarray(m[n]) for m in in_maps], axis=0)
        for n in in_names
    ]
    concat_zeros = [
        np.zeros((8 * z.shape[0], *z.shape[1:]), z.dtype) for z in zero_outs
    ]
    sharding = jax.sharding.NamedSharding(mesh, spec)
    dev_args = [jax.device_put(a, sharding) for a in concat_in + concat_zeros]
    r = f(*dev_args)
    jax.block_until_ready(r)
    best = None
    for _ in range(reps):
        t0 = _time.perf_counter()
        r = f(*dev_args)
        jax.block_until_ready(r)
        dt = _time.perf_counter() - t0
        best = dt if best is None else min(best, dt)
    return int(best * 1e9)


# revision 33
# speedup vs baseline: 1.0282x; 1.0043x over previous
"""Trainium2 Bass kernel for GQA attention (nn_Attention_34832184770944).

Sharding: tensor-parallel across heads on 8 cores. Core m gets KV head m and
Q heads 4m..4m+3: wq/wk/wv sharded column-wise, wo row-wise. Each core
computes a full-shape bf16 partial output; the host sums the 8 partials in
fp32.

Device kernel design (v2, restructured from the 478us baseline):
  - QKV projection computed e-major (lhsT = weight tile, rhs = xT chunk of
    512 tokens) so Q and K come out pre-transposed; no PE transpose pass.
  - Host permutes wq/wk columns within each head (even indices first) so
    RoPE becomes NeoX rotate-half: partition-shift DVE copies + full-width
    bf16 tensor_tensor ops per chunk.
  - V is projected e-major then PE-transposed to token-major; each 128-col
    V block is [vT (64) | ones (64)], so the PV matmul emits softmax
    denominators REPLICATED on PSUM partitions 64:128 at zero extra cost
    (matmul time is set by rhs streaming, not output rows).
  - Normalize fused out of PSUM: reciprocal_approx_fast on rows 64:128 then
    one tensor_mul (PSUM rows 0:64 x rinv -> oT bf16). No DRAM round trip,
    no 1-partition reciprocal, no broadcast matmul, no separate oU copy.
  - Scores per (key tile, head pair): two concurrent row-group matmuls
    (heads at PE row strips 0/64 via KR duplicated into both halves) write
    one [128,1024] fp32 2-bank PSUM tile; ONE batched exp per pair tile
    (amortizes the ~352-cycle ACT instruction overhead).
  - Causal trimming on diagonal tiles: scores/exp/PV restricted to
    queries >= 128*d; the in-block triangle is zeroed on pt with a gpsimd
    affine_select (no DVE mask adds, no pt memsets).
  - exp on ScalarE straight out of PSUM with the 1/8 scale folded in; no
    max-subtraction (|scores|/8 stays well inside fp32 exp range).
  - Emission interleaves attention phases of chunk k-1 with the projection
    chains of chunk k so the PE instruction stream always has matmuls
    between exp-dependent PV steps (keeps the PE HAM clock-gate warm).
  - Weight/freq DMAs spread over the scalar/vector/gpsimd queues, x chunks
    and output tiles on the sync queue.
"""

import os
import sys

sys.path.insert(0, "/opt/trn_rl_repo")

import numpy as np
import ml_dtypes

BF16 = ml_dtypes.bfloat16

B, S, D = 2, 2048, 2048
NH = 4              # q heads per core
HD = 64             # head dim
KD = D // 128       # 16 contraction tiles
TT = S // 128       # 16 token tiles per batch
NCH = S // 512      # 4 query chunks per batch
SCALE = 1.0 / 8.0


def _build_bass():
    import functools

    import concourse.bacc as bacc
    import concourse.mybir as mybir
    from concourse.tile import TileContext
    from concourse.masks import make_identity

    # This kernel uses Exp (softmax) and Ln (1/d = exp(-ln d)) on ScalarE.
    # The act-table pass maps Exp -> "exp_and_others" and Ln ->
    # "natural_log", thrashing the 2.7us table load between them.  Narrow
    # the candidate sets (set names/indices preserved) so both resolve to
    # "natural_log_exp_and_others", which contains exp, ln AND copy ->
    # exactly one table load for the whole kernel.  Patch is scoped to this
    # build and restored afterwards.
    _orig_gat = bacc.get_activation_tables
    Exp_f = mybir.ActivationFunctionType.Exp
    Ln_f = mybir.ActivationFunctionType.Ln

    @functools.wraps(_orig_gat)
    def _gat(arch):
        tables = dict(_orig_gat(arch))
        out = {}
        for name, fns in tables.items():
            if name != "natural_log_exp_and_others":
                fns = fns - {Exp_f, Ln_f}
            out[name] = fns
        return out

    bacc.get_activation_tables = _gat

    f32 = mybir.dt.float32
    bf16 = mybir.dt.bfloat16
    Exp = mybir.ActivationFunctionType.Exp
    Copy = mybir.ActivationFunctionType.Copy
    Log = mybir.ActivationFunctionType.Ln

    nc = bacc.Bacc(None, target_bir_lowering=False)
    xT_d = nc.dram_tensor("xT", [B, D, S], bf16, kind="ExternalInput")
    wqkv_d = nc.dram_tensor("wqkv", [D, 384], bf16, kind="ExternalInput")
    wo01_d = nc.dram_tensor("wo01", [128, D], bf16, kind="ExternalInput")
    wo23_d = nc.dram_tensor("wo23", [128, D], bf16, kind="ExternalInput")
    cq_d = nc.dram_tensor("cq", [128, S], bf16, kind="ExternalInput")
    sq_d = nc.dram_tensor("sq", [128, S], bf16, kind="ExternalInput")
    out_d = nc.dram_tensor("out", [B, S, D], bf16, kind="ExternalOutput")

    with TileContext(nc) as tc:
        with (
            tc.tile_pool(name="const", bufs=1) as constp,
            tc.tile_pool(name="wts", bufs=1) as wtsp,
            tc.tile_pool(name="xin", bufs=2) as xinp,
            tc.tile_pool(name="kv8", bufs=10) as kv8p,
            tc.tile_pool(name="qch", bufs=6) as qchp,
            tc.tile_pool(name="och", bufs=6) as ochp,
            tc.tile_pool(name="qw", bufs=4) as qwp,
            tc.tile_pool(name="sh", bufs=3) as shp,
            tc.tile_pool(name="rt", bufs=4) as rtp,
            tc.tile_pool(name="pt", bufs=20) as ptp,
            tc.tile_pool(name="ri", bufs=6) as rip,
            tc.tile_pool(name="ost", bufs=3) as ostp,
            tc.tile_pool(name="pj", bufs=2, space="PSUM") as pjp,
            tc.tile_pool(name="sc", bufs=2, space="PSUM") as scp,
            tc.tile_pool(name="pv", bufs=2, space="PSUM") as pvp,
        ):
            # ---- constants ----
            ident = constp.tile([128, 128], bf16, name="ident")
            make_identity(nc, ident[:, :])

            # ---- weights / freqs spread across idle DMA queues so x chunk
            # 0 (sync queue) and wqkv (scalar queue) load in parallel ----
            # wqkv in four quarters so the first projection matmuls can
            # start as soon as the first weight slice lands
            NSP = 4
            KQ = KD // NSP
            wqkv_sbs = []
            for h in range(NSP):
                wq_h = wtsp.tile([128, KQ * 384], bf16, name=f"wqkv_sb{h}")
                nc.scalar.dma_start(
                    out=wq_h[:, :].rearrange("p (k e) -> p k e", k=KQ),
                    in_=wqkv_d.rearrange("(k p) e -> p k e", p=128)[
                        :, h * KQ : (h + 1) * KQ
                    ],
                )
                wqkv_sbs.append(wq_h)
            cq_sb = wtsp.tile([128, S], bf16, name="cq_sb")
            nc.scalar.dma_start(out=cq_sb[:, :], in_=cq_d[:, :])
            sq_sb = wtsp.tile([128, S], bf16, name="sq_sb")
            nc.scalar.dma_start(out=sq_sb[:, :], in_=sq_d[:, :])

            # per-(batch,chunk) tile registries
            Xc = {}    # (b,c) -> [128, KD*512] x chunk (e-major)
            KRc = {}   # (b,c) -> [128,512] rotated K duplicated both halves
            Vc = {}    # (b,c) -> [128, 4*128] token-major [vT | ones] blocks
            QRc = {}   # (b,c) -> (QR01, QR23)
            OTc = {}   # (b,c) -> (oT01, oT23) normalized outputs

            def ensure_x(b, c):
                if (b, c) in Xc:
                    return
                csl = slice(c * 512, c * 512 + 512)
                halves = []
                for h in range(NSP):
                    xh = xinp.tile(
                        [128, KQ * 512], bf16, tag=f"xc{h}", name=f"x{h}"
                    )
                    nc.sync.dma_start(
                        out=xh[:, :].rearrange("p (k t) -> p k t", k=KQ),
                        in_=xT_d[b, :, csl].rearrange(
                            "(k p) t -> p k t", p=128
                        )[:, h * KQ : (h + 1) * KQ],
                    )
                    halves.append(xh)
                Xc[(b, c)] = halves

            def emit_proj_chain(b, c, which):
                """One of three projection chains for chunk (b,c):
                which=0: q01 proj + rope; which=1: q23 proj + rope;
                which=2: kv proj + K rope + V transpose."""
                ensure_x(b, c)
                csl = slice(c * 512, c * 512 + 512)
                xh = Xc[(b, c)]
                et = which
                ps = pjp.tile([128, 512], f32, tag="pj", name="ps_prj")
                for kd in range(KD):
                    h, kh = divmod(kd, KQ)
                    nc.tensor.matmul(
                        ps[:, :],
                        lhsT=wqkv_sbs[h][
                            :, kh * 384 + et * 128 : kh * 384 + et * 128 + 128
                        ],
                        rhs=xh[h][:, kh * 512 : kh * 512 + 512],
                        start=(kd == 0),
                        stop=(kd == KD - 1),
                    )
                raw = qwp.tile([128, 512], bf16, tag="qraw", name="raw")
                if which == 0:
                    nc.scalar.activation(raw[:, :], ps[:, :], Copy)
                else:
                    nc.vector.tensor_copy(raw[:, :], ps[:, :])

                if which < 2:
                    # rope for a q pair
                    QR = qchp.tile([128, 512], bf16, tag="qrc", name="QR")
                    qsh = shp.tile([128, 512], bf16, tag="sh", name="qsh")
                    for blk in range(4):
                        src = (blk ^ 1) * 32
                        nc.vector.tensor_copy(
                            qsh[blk * 32 : blk * 32 + 32, :],
                            raw[src : src + 32, :],
                        )
                    t1 = rtp.tile([128, 512], bf16, tag="rt", name="t1")
                    t2 = rtp.tile([128, 512], bf16, tag="rt", name="t2")
                    nc.vector.tensor_mul(t1[:, :], raw[:, :], cq_sb[:, csl])
                    nc.vector.tensor_mul(t2[:, :], qsh[:, :], sq_sb[:, csl])
                    nc.vector.tensor_add(QR[:, :], t1[:, :], t2[:, :])
                    QRc.setdefault((b, c), {})[which] = QR
                    return

                # which == 2: V -> token-major [vT | ones] blocks
                v_sb = kv8p.tile([128, 4 * 128], bf16, tag="vc", name="v_sb")
                nc.gpsimd.memset(v_sb[:, :], 1.0)
                for ts in range(4):
                    ps_t = pjp.tile([128, 64], bf16, tag="pj", name="ps_t")
                    nc.tensor.transpose(
                        ps_t[:, 0:64],
                        raw[0:64, ts * 128 : ts * 128 + 128],
                        ident[0:64, 0:64],
                    )
                    nc.vector.tensor_copy(
                        v_sb[:, ts * 128 : ts * 128 + 64], ps_t[:, 0:64]
                    )
                Vc[(b, c)] = v_sb

                # rope for k (rows 64:128 of raw), duplicated into both
                # partition halves of KR
                KR = kv8p.tile([128, 512], bf16, tag="krc", name="KR")
                ksh = shp.tile([128, 512], bf16, tag="sh", name="ksh")
                nc.vector.tensor_copy(ksh[64:96, :], raw[96:128, :])
                nc.vector.tensor_copy(ksh[96:128, :], raw[64:96, :])
                k1 = rtp.tile([128, 512], bf16, tag="rt", name="k1")
                k2 = rtp.tile([128, 512], bf16, tag="rt", name="k2")
                nc.vector.tensor_mul(
                    k1[0:64, :], raw[64:128, :], cq_sb[64:128, csl]
                )
                nc.vector.tensor_mul(
                    k2[0:64, :], ksh[64:128, :], sq_sb[64:128, csl]
                )
                nc.vector.tensor_add(KR[0:64, :], k1[0:64, :], k2[0:64, :])
                nc.vector.tensor_add(KR[64:128, :], k1[0:64, :], k2[0:64, :])
                KRc[(b, c)] = KR

            def emit_scores(b, j, hp, pts):
                """scores + batched exp for query chunk j, head pair hp."""
                nts = 4 * j + 4
                QR = QRc[(b, j)][hp]
                for i in range(nts):
                    dd = i - 4 * j
                    trim = 128 * max(dd, 0)
                    KR = KRc[(b, i // 4)]
                    ksl = slice((i % 4) * 128, (i % 4) * 128 + 128)
                    ps_s = scp.tile([128, 1024], f32, tag="sc", name="ps_s")
                    for sub in range(2):
                        r0 = sub * 64
                        nc.tensor.matmul(
                            ps_s[:, sub * 512 + trim : sub * 512 + 512],
                            lhsT=KR[r0 : r0 + 64, ksl],
                            rhs=QR[r0 : r0 + 64, trim:512],
                            start=True,
                            stop=True,
                        )
                    pt = ptp.tile([128, 1024], bf16, tag="pt", name="pt")
                    if trim == 0:
                        nc.scalar.activation(
                            pt[:, :], ps_s[:, :], Exp, scale=SCALE
                        )
                    else:
                        ps3 = ps_s[:, :].rearrange("p (h t) -> p h t", h=2)
                        pt3 = pt[:, :].rearrange("p (h t) -> p h t", h=2)
                        nc.scalar.activation(
                            pt3[:, :, trim:512],
                            ps3[:, :, trim:512],
                            Exp,
                            scale=SCALE,
                        )
                    if dd >= 0:
                        if trim > 0:
                            # zero the fully-masked query range skipped by
                            # the trimmed exp (PV reads the full tile)
                            pt3z = pt[:, :].rearrange(
                                "p (h t) -> p h t", h=2
                            )
                            nc.gpsimd.memset(pt3z[:, :, 0:trim], 0.0)
                        # zero the in-block causal triangle (q < k)
                        for h2 in range(2):
                            blk = slice(
                                h2 * 512 + trim, h2 * 512 + trim + 128
                            )
                            nc.gpsimd.affine_select(
                                out=pt[:, blk],
                                in_=pt[:, blk],
                                compare_op=mybir.AluOpType.is_ge,
                                fill=0.0,
                                base=0,
                                pattern=[[1, 128]],
                                channel_multiplier=-1,
                            )
                    pts[(i, hp)] = pt

            def emit_pv(b, j, hp, pts):
                """PV chains + fused normalize for head pair hp of chunk j."""
                nts = 4 * j + 4
                oT = ochp.tile([128, 512], bf16, tag="otc", name="oT")
                for sub in range(2):
                    r0 = sub * 64
                    ps_pv = pvp.tile([128, 512], f32, tag="pv", name="ps_pv")
                    for i in range(nts):
                        v_sb = Vc[(b, i // 4)]
                        vsl = slice((i % 4) * 128, (i % 4) * 128 + 128)
                        nc.tensor.matmul(
                            ps_pv[:, :],
                            lhsT=v_sb[:, vsl],
                            rhs=pts[(i, hp)][:, sub * 512 : sub * 512 + 512],
                            start=(i == 0),
                            stop=(i == nts - 1),
                        )
                    # 1/d = exp(-ln(d)) on ScalarE: Log and Exp share the
                    # natural_log_exp_and_others table set (no switch cost)
                    lnd = rip.tile([64, 512], f32, tag="ln", name="lnd")
                    rinv = rip.tile([64, 512], f32, tag="ri", name="rinv")
                    nc.scalar.activation(
                        lnd[:, :], ps_pv[64:128, :], Log
                    )
                    nc.scalar.activation(
                        rinv[:, :], lnd[:, :], Exp, scale=-1.0
                    )
                    nc.vector.tensor_mul(
                        oT[r0 : r0 + 64, :], ps_pv[0:64, :], rinv[:, :]
                    )
                OTc.setdefault((b, j), {})[hp] = oT

            def emit_outproj(b, j):
                """output projection for the 4 token tiles of chunk j."""
                oT01 = OTc[(b, j)][0]
                oT23 = OTc[(b, j)][1]
                for ts in range(4):
                    tt = j * 4 + ts
                    tsl = slice(ts * 128, ts * 128 + 128)
                    ot = ostp.tile([128, D], bf16, tag="ot", name="ot")
                    for dmc in range(4):
                        po = pvp.tile([128, 512], f32, tag="pv", name="po")
                        nc.tensor.matmul(
                            po[:, :],
                            lhsT=oT01[:, tsl],
                            rhs=wo01_sb[:, dmc * 512 : dmc * 512 + 512],
                            start=True,
                            stop=False,
                        )
                        nc.tensor.matmul(
                            po[:, :],
                            lhsT=oT23[:, tsl],
                            rhs=wo23_sb[:, dmc * 512 : dmc * 512 + 512],
                            start=False,
                            stop=True,
                        )
                        if dmc == 0:
                            nc.scalar.activation(
                                ot[:, dmc * 512 : dmc * 512 + 512],
                                po[:, :],
                                Copy,
                            )
                        else:
                            nc.vector.tensor_copy(
                                ot[:, dmc * 512 : dmc * 512 + 512], po[:, :]
                            )
                    nc.sync.dma_start(
                        out=out_d[b, tt * 128 : tt * 128 + 128, :],
                        in_=ot[:, :],
                    )

            # emission: interleave attention phases of chunk k-1 with the
            # projection chains of chunk k so the PE stream always has
            # exp-independent matmuls to fill exp-wait gaps.
            chunks = [(b, c) for b in range(B) for c in range(NCH)]
            ensure_x(*chunks[0])
            # wo loads queue on sync AFTER the first x chunk (needed ~40us
            # in, while x0 gates the very first projection matmul)
            wo01_sb = wtsp.tile([128, D], bf16, name="wo01_sb")
            nc.sync.dma_start(out=wo01_sb[:, :], in_=wo01_d[:, :])
            wo23_sb = wtsp.tile([128, D], bf16, name="wo23_sb")
            nc.sync.dma_start(out=wo23_sb[:, :], in_=wo23_d[:, :])
            for idx, (b, c) in enumerate(chunks):
                A = chunks[idx - 1] if idx >= 1 else None
                O = chunks[idx - 2] if idx >= 2 else None
                pts = {}
                if A is not None:
                    emit_scores(A[0], A[1], 0, pts)
                emit_proj_chain(b, c, 0)
                if idx + 1 < len(chunks):
                    ensure_x(*chunks[idx + 1])
                if O is not None:
                    # chunk idx-2's output projection: exp-independent PE
                    # filler placed inside chunk idx-1's exp-wait window
                    emit_outproj(O[0], O[1])
                if A is not None:
                    emit_pv(A[0], A[1], 0, pts)
                emit_proj_chain(b, c, 1)
                if A is not None:
                    emit_scores(A[0], A[1], 1, pts)
                emit_proj_chain(b, c, 2)
                if A is not None:
                    emit_pv(A[0], A[1], 1, pts)
            # tail: attention + outproj of the final chunks
            A = chunks[-1]
            pts = {}
            emit_scores(A[0], A[1], 0, pts)
            emit_outproj(*chunks[-2])
            emit_pv(A[0], A[1], 0, pts)
            emit_scores(A[0], A[1], 1, pts)
            emit_pv(A[0], A[1], 1, pts)
            emit_outproj(A[0], A[1])
    try:
        nc.compile()
    finally:
        bacc.get_activation_tables = _orig_gat
    return nc


def _prep_in_maps(inputs):
    x = np.asarray(inputs["x"], dtype=np.float32)
    fc = np.asarray(inputs["freqs_cos"], dtype=np.float32)
    fs = np.asarray(inputs["freqs_sin"], dtype=np.float32)
    wq = np.asarray(inputs["wq"], dtype=np.float32)
    wk = np.asarray(inputs["wk"], dtype=np.float32)
    wv = np.asarray(inputs["wv"], dtype=np.float32)
    wo = np.asarray(inputs["wo"], dtype=np.float32)

    xT = np.ascontiguousarray(np.transpose(x, (0, 2, 1))).astype(BF16)
    c = np.ascontiguousarray(fc.T)  # [32, S]
    s = np.ascontiguousarray(fs.T)
    cq = np.concatenate([c, c, c, c], axis=0).astype(BF16)      # [128, S]
    sq = np.concatenate([-s, s, -s, s], axis=0).astype(BF16)    # [128, S]
    perm = np.concatenate([np.arange(0, 64, 2), np.arange(1, 64, 2)])

    in_maps = []
    for m in range(8):
        wqs = wq[:, m * 256 : m * 256 + 256].reshape(D, 4, 64)[:, :, perm]
        wq01 = wqs[:, 0:2].reshape(D, 128)
        wq23 = wqs[:, 2:4].reshape(D, 128)
        wks = wk[:, m * 64 : m * 64 + 64][:, perm]
        wvs = wv[:, m * 64 : m * 64 + 64]
        wkv = np.concatenate([wvs, wks], axis=1)  # vT rows 0:64, k rows 64:128
        wqkv = np.ascontiguousarray(
            np.concatenate([wq01, wq23, wkv], axis=1)
        ).astype(BF16)
        wo01 = np.ascontiguousarray(wo[m * 256 : m * 256 + 128, :]).astype(BF16)
        wo23 = np.ascontiguousarray(wo[m * 256 + 128 : m * 256 + 256, :]).astype(
            BF16
        )
        in_maps.append(
            dict(xT=xT, wqkv=wqkv, wo01=wo01, wo23=wo23, cq=cq, sq=sq)
        )
    return in_maps


LAST_EXEC_NS = None


def _install_ntff_hook():
    """Provide antenv.axon_hooks (missing in some containers) so that
    run_bass_kernel_spmd(trace=True) can capture an NTFF profile."""
    import types

    try:
        import antenv.axon_hooks  # noqa: F401
        return True
    except ImportError:
        pass
    try:
        import antenv
        from trn_agent_boot.trn_boot import _ntff_profile_via_ctypes

        hook = _ntff_profile_via_ctypes("/opt/axon/libaxon_pjrt.so")
        if hook is None:
            return False
        mod = types.ModuleType("antenv.axon_hooks")
        mod._hook = hook
        mod.set_axon_ntff_profile_hook = lambda h: setattr(mod, "_hook", h)
        mod.get_axon_ntff_profile_hook = lambda: mod._hook
        sys.modules["antenv.axon_hooks"] = mod
        antenv.axon_hooks = mod
        return True
    except Exception:
        return False


def kernel(**inputs):
    global LAST_EXEC_NS
    from concourse import bass_utils

    in_maps = _prep_in_maps(inputs)
    nc = _build_bass()
    trace = bool(int(os.environ.get("KERNEL_TRACE", "0")))
    if trace:
        trace = _install_ntff_hook()
    res = bass_utils.run_bass_kernel_spmd(
        nc, in_maps, core_ids=list(range(8)), trace=trace
    )
    if trace and res.exec_time_ns is not None:
        LAST_EXEC_NS = res.exec_time_ns
    out = np.zeros((B, S, D), dtype=np.float32)
    for r in res.results:
        out += r["out"].astype(np.float32)
    return out


def time_device(reps=6, **inputs):
    """Wall-clock the sharded PJRT executable with device-resident inputs
    (fallback when NTFF profiling is unavailable; includes axon dispatch
    overhead)."""
    import jax
    from concourse import bass2jax
    import concourse.mybir as mybir
    import time as _time

    in_maps = _prep_in_maps(inputs)
    nc = _build_bass()
    bass2jax.install_neuronx_cc_hook()

    partition_name = (
        nc.partition_id_tensor.name if nc.partition_id_tensor else None
    )
    in_names, out_names, out_avals, zero_outs = [], [], [], []
    for alloc in nc.m.functions[0].allocations:
        if not isinstance(alloc, mybir.MemoryLocationSet):
            continue
        name = alloc.memorylocations[0].name
        if alloc.kind == "ExternalInput":
            if name != partition_name:
                in_names.append(name)
        elif alloc.kind == "ExternalOutput":
            out_names.append(name)
            shape = tuple(alloc.tensor_shape)
            dt = mybir.dt.np(alloc.dtype)
            out_avals.append(jax.core.ShapedArray(shape, dt))
            zero_outs.append(np.zeros(shape, dt))
    n_params = len(in_names)
    in_all = in_names + out_names
    if partition_name is not None:
        in_all = in_all + [partition_name]

    def _body(*args):
        operands = list(args)
        if partition_name is not None:
            operands.append(bass2jax.partition_id_tensor())
        outs = bass2jax._bass_exec_p.bind(
            *operands,
            out_avals=tuple(out_avals),
            in_names=tuple(in_all),
            out_names=tuple(out_names),
            lowering_input_output_aliases=(),
            sim_require_finite=True,
            sim_require_nnan=True,
            nc=nc,
        )
        return tuple(outs)

    devices = jax.devices()[:8]
    mesh = bass2jax.Mesh(np.asarray(devices), ("core",))
    spec = bass2jax.PartitionSpec("core")
    nin = n_params + len(out_names)
    f = jax.jit(
        bass2jax.shard_map(
            _body,
            mesh=mesh,
            in_specs=(spec,) * nin,
            out_specs=(spec,) * len(out_names),
            check_rep=False,
        )
    )
    concat_in = [
        np.concatenate([np.asarray(m[n]) for m in in_maps], axis=0)
        for n in in_names
    ]
    concat_zeros = [
        np.zeros((8 * z.shape[0], *z.shape[1:]), z.dtype) for z in zero_outs
    ]
    sharding = jax.sharding.NamedSharding(mesh, spec)
    dev_args = [jax.device_put(a, sharding) for a in concat_in + concat_zeros]
    r = f(*dev_args)
    jax.block_until_ready(r)
    best = None
    for _ in range(reps):
        t0 = _time.perf_counter()
        r = f(*dev_args)
        jax.block_until_ready(r)
        dt = _time.perf_counter() - t0
        best = dt if best is None else min(best, dt)
    return int(best * 1e9)


# revision 40
# speedup vs baseline: 1.0472x; 1.0184x over previous
"""Trainium2 Bass kernel for GQA attention (nn_Attention_34832184770944).

Sharding: tensor-parallel across heads on 8 cores. Core m gets KV head m and
Q heads 4m..4m+3: wq/wk/wv sharded column-wise, wo row-wise. Each core
computes a full-shape bf16 partial output; the host sums the 8 partials in
fp32.

Device kernel design (v2, restructured from the 478us baseline):
  - QKV projection computed e-major (lhsT = weight tile, rhs = xT chunk of
    512 tokens) so Q and K come out pre-transposed; no PE transpose pass.
  - Host permutes wq/wk columns within each head (even indices first) so
    RoPE becomes NeoX rotate-half: partition-shift DVE copies + full-width
    bf16 tensor_tensor ops per chunk.
  - V is projected e-major then PE-transposed to token-major; each 128-col
    V block is [vT (64) | ones (64)], so the PV matmul emits softmax
    denominators REPLICATED on PSUM partitions 64:128 at zero extra cost
    (matmul time is set by rhs streaming, not output rows).
  - Normalize fused out of PSUM: reciprocal_approx_fast on rows 64:128 then
    one tensor_mul (PSUM rows 0:64 x rinv -> oT bf16). No DRAM round trip,
    no 1-partition reciprocal, no broadcast matmul, no separate oU copy.
  - Scores per (key tile, head pair): two concurrent row-group matmuls
    (heads at PE row strips 0/64 via KR duplicated into both halves) write
    one [128,1024] fp32 2-bank PSUM tile; ONE batched exp per pair tile
    (amortizes the ~352-cycle ACT instruction overhead).
  - Causal trimming on diagonal tiles: scores/exp/PV restricted to
    queries >= 128*d; the in-block triangle is zeroed on pt with a gpsimd
    affine_select (no DVE mask adds, no pt memsets).
  - exp on ScalarE straight out of PSUM with the 1/8 scale folded in; no
    max-subtraction (|scores|/8 stays well inside fp32 exp range).
  - Emission interleaves attention phases of chunk k-1 with the projection
    chains of chunk k so the PE instruction stream always has matmuls
    between exp-dependent PV steps (keeps the PE HAM clock-gate warm).
  - Weight/freq DMAs spread over the scalar/vector/gpsimd queues, x chunks
    and output tiles on the sync queue.
"""

import os
import sys

sys.path.insert(0, "/opt/trn_rl_repo")

import numpy as np
import ml_dtypes

BF16 = ml_dtypes.bfloat16

B, S, D = 2, 2048, 2048
NH = 4              # q heads per core
HD = 64             # head dim
KD = D // 128       # 16 contraction tiles
TT = S // 128       # 16 token tiles per batch
NCH = S // 512      # 4 query chunks per batch
SCALE = 1.0 / 8.0

# Trim PV matmuls to the causally-live query range on diagonal key tiles.
# HW-correct (per-element has_written semantics; validated byte-identical
# vs full-range accumulation on device) but CoreSim's accumulation-group
# model rejects partial-range groups -- debug_sim.py sets this False.
PV_TRIM = True


def _build_bass():
    import functools

    import concourse.bacc as bacc
    import concourse.mybir as mybir
    from concourse.tile import TileContext
    from concourse.masks import make_identity

    # This kernel uses Exp (softmax) and Ln (1/d = exp(-ln d)) on ScalarE.
    # The act-table pass maps Exp -> "exp_and_others" and Ln ->
    # "natural_log", thrashing the 2.7us table load between them.  Narrow
    # the candidate sets (set names/indices preserved) so both resolve to
    # "natural_log_exp_and_others", which contains exp, ln AND copy ->
    # exactly one table load for the whole kernel.  Patch is scoped to this
    # build and restored afterwards.
    _orig_gat = bacc.get_activation_tables
    Exp_f = mybir.ActivationFunctionType.Exp
    Ln_f = mybir.ActivationFunctionType.Ln

    @functools.wraps(_orig_gat)
    def _gat(arch):
        tables = dict(_orig_gat(arch))
        out = {}
        for name, fns in tables.items():
            if name != "natural_log_exp_and_others":
                fns = fns - {Exp_f, Ln_f}
            out[name] = fns
        return out

    bacc.get_activation_tables = _gat

    f32 = mybir.dt.float32
    bf16 = mybir.dt.bfloat16
    Exp = mybir.ActivationFunctionType.Exp
    Copy = mybir.ActivationFunctionType.Copy
    Log = mybir.ActivationFunctionType.Ln

    nc = bacc.Bacc(None, target_bir_lowering=False)
    xT_d = nc.dram_tensor("xT", [B, D, S], bf16, kind="ExternalInput")
    wqkv_d = nc.dram_tensor("wqkv", [D, 384], bf16, kind="ExternalInput")
    wo01_d = nc.dram_tensor("wo01", [128, D], bf16, kind="ExternalInput")
    wo23_d = nc.dram_tensor("wo23", [128, D], bf16, kind="ExternalInput")
    cq_d = nc.dram_tensor("cq", [128, S], bf16, kind="ExternalInput")
    sq_d = nc.dram_tensor("sq", [128, S], bf16, kind="ExternalInput")
    out_d = nc.dram_tensor("out", [B, S, D], bf16, kind="ExternalOutput")

    with TileContext(nc) as tc:
        with (
            tc.tile_pool(name="const", bufs=1) as constp,
            tc.tile_pool(name="wts", bufs=1) as wtsp,
            tc.tile_pool(name="xin", bufs=2) as xinp,
            tc.tile_pool(name="kv8", bufs=10) as kv8p,
            tc.tile_pool(name="qch", bufs=6) as qchp,
            tc.tile_pool(name="och", bufs=6) as ochp,
            tc.tile_pool(name="qw", bufs=4) as qwp,
            tc.tile_pool(name="sh", bufs=3) as shp,
            tc.tile_pool(name="rt", bufs=4) as rtp,
            tc.tile_pool(name="pt", bufs=20) as ptp,
            tc.tile_pool(name="ri", bufs=6) as rip,
            tc.tile_pool(name="ost", bufs=3) as ostp,
            tc.tile_pool(name="pj", bufs=2, space="PSUM") as pjp,
            tc.tile_pool(name="sc", bufs=2, space="PSUM") as scp,
            tc.tile_pool(name="pv", bufs=2, space="PSUM") as pvp,
        ):
            # ---- constants ----
            ident = constp.tile([128, 128], bf16, name="ident")
            make_identity(nc, ident[:, :])

            # ---- weights / freqs spread across idle DMA queues so x chunk
            # 0 (sync queue) and wqkv (scalar queue) load in parallel ----
            # wqkv in four quarters so the first projection matmuls can
            # start as soon as the first weight slice lands
            NSP = 4
            KQ = KD // NSP
            NSPX = 8
            KQX = KD // NSPX
            wqkv_sbs = []
            for h in range(NSP):
                wq_h = wtsp.tile([128, KQ * 384], bf16, name=f"wqkv_sb{h}")
                nc.scalar.dma_start(
                    out=wq_h[:, :].rearrange("p (k e) -> p k e", k=KQ),
                    in_=wqkv_d.rearrange("(k p) e -> p k e", p=128)[
                        :, h * KQ : (h + 1) * KQ
                    ],
                )
                wqkv_sbs.append(wq_h)
            cq_sb = wtsp.tile([128, S], bf16, name="cq_sb")
            nc.scalar.dma_start(out=cq_sb[:, :], in_=cq_d[:, :])
            sq_sb = wtsp.tile([128, S], bf16, name="sq_sb")
            nc.scalar.dma_start(out=sq_sb[:, :], in_=sq_d[:, :])

            # per-(batch,chunk) tile registries
            Xc = {}    # (b,c) -> [128, KD*512] x chunk (e-major)
            KRc = {}   # (b,c) -> [128,512] rotated K duplicated both halves
            Vc = {}    # (b,c) -> [128, 4*128] token-major [vT | ones] blocks
            QRc = {}   # (b,c) -> (QR01, QR23)
            OTc = {}   # (b,c) -> (oT01, oT23) normalized outputs

            def ensure_x(b, c):
                if (b, c) in Xc:
                    return
                csl = slice(c * 512, c * 512 + 512)
                halves = []
                for h in range(NSPX):
                    xh = xinp.tile(
                        [128, KQX * 512], bf16, tag=f"xc{h}", name=f"x{h}"
                    )
                    nc.sync.dma_start(
                        out=xh[:, :].rearrange("p (k t) -> p k t", k=KQX),
                        in_=xT_d[b, :, csl].rearrange(
                            "(k p) t -> p k t", p=128
                        )[:, h * KQX : (h + 1) * KQX],
                    )
                    halves.append(xh)
                Xc[(b, c)] = halves

            def emit_proj_chain(b, c, which):
                """One of three projection chains for chunk (b,c):
                which=0: q01 proj + rope; which=1: q23 proj + rope;
                which=2: kv proj + K rope + V transpose."""
                ensure_x(b, c)
                csl = slice(c * 512, c * 512 + 512)
                xh = Xc[(b, c)]
                et = which
                ps = pjp.tile([128, 512], f32, tag="pj", name="ps_prj")
                for kd in range(KD):
                    hw_, kw = divmod(kd, KQ)
                    hx, kx = divmod(kd, KQX)
                    nc.tensor.matmul(
                        ps[:, :],
                        lhsT=wqkv_sbs[hw_][
                            :, kw * 384 + et * 128 : kw * 384 + et * 128 + 128
                        ],
                        rhs=xh[hx][:, kx * 512 : kx * 512 + 512],
                        start=(kd == 0),
                        stop=(kd == KD - 1),
                    )
                raw = qwp.tile([128, 512], bf16, tag="qraw", name="raw")
                if which == 0:
                    nc.scalar.activation(raw[:, :], ps[:, :], Copy)
                else:
                    nc.vector.tensor_copy(raw[:, :], ps[:, :])

                if which < 2:
                    # rope for a q pair
                    QR = qchp.tile([128, 512], bf16, tag="qrc", name="QR")
                    qsh = shp.tile([128, 512], bf16, tag="sh", name="qsh")
                    for blk in range(4):
                        src = (blk ^ 1) * 32
                        nc.vector.tensor_copy(
                            qsh[blk * 32 : blk * 32 + 32, :],
                            raw[src : src + 32, :],
                        )
                    t1 = rtp.tile([128, 512], bf16, tag="rt", name="t1")
                    t2 = rtp.tile([128, 512], bf16, tag="rt", name="t2")
                    nc.vector.tensor_mul(t1[:, :], raw[:, :], cq_sb[:, csl])
                    nc.vector.tensor_mul(t2[:, :], qsh[:, :], sq_sb[:, csl])
                    nc.vector.tensor_add(QR[:, :], t1[:, :], t2[:, :])
                    QRc.setdefault((b, c), {})[which] = QR
                    return

                # which == 2: V -> token-major [vT | ones] blocks
                v_sb = kv8p.tile([128, 4 * 128], bf16, tag="vc", name="v_sb")
                nc.gpsimd.memset(v_sb[:, :], 1.0)
                for ts in range(4):
                    ps_t = pjp.tile([128, 64], bf16, tag="pj", name="ps_t")
                    nc.tensor.transpose(
                        ps_t[:, 0:64],
                        raw[0:64, ts * 128 : ts * 128 + 128],
                        ident[0:64, 0:64],
                    )
                    nc.vector.tensor_copy(
                        v_sb[:, ts * 128 : ts * 128 + 64], ps_t[:, 0:64]
                    )
                Vc[(b, c)] = v_sb

                # rope for k (rows 64:128 of raw), duplicated into both
                # partition halves of KR
                KR = kv8p.tile([128, 512], bf16, tag="krc", name="KR")
                ksh = shp.tile([128, 512], bf16, tag="sh", name="ksh")
                nc.vector.tensor_copy(ksh[64:96, :], raw[96:128, :])
                nc.vector.tensor_copy(ksh[96:128, :], raw[64:96, :])
                k1 = rtp.tile([128, 512], bf16, tag="rt", name="k1")
                k2 = rtp.tile([128, 512], bf16, tag="rt", name="k2")
                nc.vector.tensor_mul(
                    k1[0:64, :], raw[64:128, :], cq_sb[64:128, csl]
                )
                nc.vector.tensor_mul(
                    k2[0:64, :], ksh[64:128, :], sq_sb[64:128, csl]
                )
                nc.vector.tensor_add(KR[0:64, :], k1[0:64, :], k2[0:64, :])
                nc.vector.tensor_add(KR[64:128, :], k1[0:64, :], k2[0:64, :])
                KRc[(b, c)] = KR

            def emit_scores(b, j, hp, pts):
                """scores + batched exp for query chunk j, head pair hp."""
                nts = 4 * j + 4
                QR = QRc[(b, j)][hp]
                for i in range(nts):
                    dd = i - 4 * j
                    trim = 128 * max(dd, 0)
                    KR = KRc[(b, i // 4)]
                    ksl = slice((i % 4) * 128, (i % 4) * 128 + 128)
                    ps_s = scp.tile([128, 1024], f32, tag="sc", name="ps_s")
                    for sub in range(2):
                        r0 = sub * 64
                        nc.tensor.matmul(
                            ps_s[:, sub * 512 + trim : sub * 512 + 512],
                            lhsT=KR[r0 : r0 + 64, ksl],
                            rhs=QR[r0 : r0 + 64, trim:512],
                            start=True,
                            stop=True,
                        )
                    pt = ptp.tile([128, 1024], bf16, tag="pt", name="pt")
                    if trim == 0:
                        nc.scalar.activation(
                            pt[:, :], ps_s[:, :], Exp, scale=SCALE
                        )
                    else:
                        ps3 = ps_s[:, :].rearrange("p (h t) -> p h t", h=2)
                        pt3 = pt[:, :].rearrange("p (h t) -> p h t", h=2)
                        nc.scalar.activation(
                            pt3[:, :, trim:512],
                            ps3[:, :, trim:512],
                            Exp,
                            scale=SCALE,
                        )
                    if dd >= 0:
                        if trim > 0 and not PV_TRIM:
                            # zero the fully-masked query range skipped by
                            # the trimmed exp (PV reads the full tile)
                            pt3z = pt[:, :].rearrange(
                                "p (h t) -> p h t", h=2
                            )
                            nc.gpsimd.memset(pt3z[:, :, 0:trim], 0.0)
                        # zero the in-block causal triangle (q < k)
                        for h2 in range(2):
                            blk = slice(
                                h2 * 512 + trim, h2 * 512 + trim + 128
                            )
                            nc.gpsimd.affine_select(
                                out=pt[:, blk],
                                in_=pt[:, blk],
                                compare_op=mybir.AluOpType.is_ge,
                                fill=0.0,
                                base=0,
                                pattern=[[1, 128]],
                                channel_multiplier=-1,
                            )
                    pts[(i, hp)] = pt

            def emit_pv(b, j, hp, pts):
                """PV chains + fused normalize for head pair hp of chunk j."""
                nts = 4 * j + 4
                oT = ochp.tile([128, 512], bf16, tag="otc", name="oT")
                for sub in range(2):
                    r0 = sub * 64
                    ps_pv = pvp.tile([128, 512], f32, tag="pv", name="ps_pv")
                    for i in range(nts):
                        dd = i - 4 * j
                        trim = 128 * max(dd, 0) if PV_TRIM else 0
                        v_sb = Vc[(b, i // 4)]
                        vsl = slice((i % 4) * 128, (i % 4) * 128 + 128)
                        nc.tensor.matmul(
                            ps_pv[:, trim:512],
                            lhsT=v_sb[:, vsl],
                            rhs=pts[(i, hp)][
                                :, sub * 512 + trim : sub * 512 + 512
                            ],
                            start=(i == 0),
                            stop=(i == nts - 1),
                            skip_group_check=(trim > 0),
                        )
                    # 1/d = exp(-ln(d)) on ScalarE: Log and Exp share the
                    # natural_log_exp_and_others table set (no switch cost)
                    lnd = rip.tile([64, 512], f32, tag="ln", name="lnd")
                    rinv = rip.tile([64, 512], f32, tag="ri", name="rinv")
                    nc.scalar.activation(
                        lnd[:, :], ps_pv[64:128, :], Log
                    )
                    nc.scalar.activation(
                        rinv[:, :], lnd[:, :], Exp, scale=-1.0
                    )
                    nc.vector.tensor_mul(
                        oT[r0 : r0 + 64, :], ps_pv[0:64, :], rinv[:, :]
                    )
                OTc.setdefault((b, j), {})[hp] = oT

            def emit_outproj(b, j):
                """output projection for the 4 token tiles of chunk j."""
                oT01 = OTc[(b, j)][0]
                oT23 = OTc[(b, j)][1]
                for ts in range(4):
                    tt = j * 4 + ts
                    tsl = slice(ts * 128, ts * 128 + 128)
                    ot = ostp.tile([128, D], bf16, tag="ot", name="ot")
                    for dmc in range(4):
                        po = pvp.tile([128, 512], f32, tag="pv", name="po")
                        nc.tensor.matmul(
                            po[:, :],
                            lhsT=oT01[:, tsl],
                            rhs=wo01_sb[:, dmc * 512 : dmc * 512 + 512],
                            start=True,
                            stop=False,
                        )
                        nc.tensor.matmul(
                            po[:, :],
                            lhsT=oT23[:, tsl],
                            rhs=wo23_sb[:, dmc * 512 : dmc * 512 + 512],
                            start=False,
                            stop=True,
                        )
                        if dmc == 0:
                            nc.scalar.activation(
                                ot[:, dmc * 512 : dmc * 512 + 512],
                                po[:, :],
                                Copy,
                            )
                        else:
                            nc.vector.tensor_copy(
                                ot[:, dmc * 512 : dmc * 512 + 512], po[:, :]
                            )
                        if dmc % 2 == 1:
                            # fire the half-row DMA as soon as its two
                            # copies land (shortens the final-store tail)
                            hsl = slice((dmc - 1) * 512, (dmc + 1) * 512)
                            nc.sync.dma_start(
                                out=out_d[b, tt * 128 : tt * 128 + 128, hsl],
                                in_=ot[:, hsl],
                            )

            # emission: interleave attention phases of chunk k-1 with the
            # projection chains of chunk k so the PE stream always has
            # exp-independent matmuls to fill exp-wait gaps.
            chunks = [(b, c) for b in range(B) for c in range(NCH)]
            ensure_x(*chunks[0])
            # wo loads queue on sync AFTER the first x chunk (needed ~40us
            # in, while x0 gates the very first projection matmul)
            wo01_sb = wtsp.tile([128, D], bf16, name="wo01_sb")
            nc.sync.dma_start(out=wo01_sb[:, :], in_=wo01_d[:, :])
            wo23_sb = wtsp.tile([128, D], bf16, name="wo23_sb")
            nc.sync.dma_start(out=wo23_sb[:, :], in_=wo23_d[:, :])
            for idx, (b, c) in enumerate(chunks):
                A = chunks[idx - 1] if idx >= 1 else None
                O = chunks[idx - 2] if idx >= 2 else None
                pts = {}
                if A is not None:
                    emit_scores(A[0], A[1], 0, pts)
                emit_proj_chain(b, c, 0)
                if idx + 1 < len(chunks):
                    ensure_x(*chunks[idx + 1])
                if O is not None:
                    # chunk idx-2's output projection: exp-independent PE
                    # filler placed inside chunk idx-1's exp-wait window
                    emit_outproj(O[0], O[1])
                if A is not None:
                    emit_pv(A[0], A[1], 0, pts)
                emit_proj_chain(b, c, 1)
                if A is not None:
                    emit_scores(A[0], A[1], 1, pts)
                emit_proj_chain(b, c, 2)
                if A is not None:
                    emit_pv(A[0], A[1], 1, pts)
            # tail: attention + outproj of the final chunks
            A = chunks[-1]
            pts = {}
            emit_scores(A[0], A[1], 0, pts)
            emit_outproj(*chunks[-2])
            emit_pv(A[0], A[1], 0, pts)
            emit_scores(A[0], A[1], 1, pts)
            emit_pv(A[0], A[1], 1, pts)
            emit_outproj(A[0], A[1])
    try:
        nc.compile()
    finally:
        bacc.get_activation_tables = _orig_gat
    return nc


def _prep_in_maps(inputs):
    x = np.asarray(inputs["x"], dtype=np.float32)
    fc = np.asarray(inputs["freqs_cos"], dtype=np.float32)
    fs = np.asarray(inputs["freqs_sin"], dtype=np.float32)
    wq = np.asarray(inputs["wq"], dtype=np.float32)
    wk = np.asarray(inputs["wk"], dtype=np.float32)
    wv = np.asarray(inputs["wv"], dtype=np.float32)
    wo = np.asarray(inputs["wo"], dtype=np.float32)

    xT = np.ascontiguousarray(np.transpose(x, (0, 2, 1))).astype(BF16)
    c = np.ascontiguousarray(fc.T)  # [32, S]
    s = np.ascontiguousarray(fs.T)
    cq = np.concatenate([c, c, c, c], axis=0).astype(BF16)      # [128, S]
    sq = np.concatenate([-s, s, -s, s], axis=0).astype(BF16)    # [128, S]
    perm = np.concatenate([np.arange(0, 64, 2), np.arange(1, 64, 2)])

    in_maps = []
    for m in range(8):
        wqs = wq[:, m * 256 : m * 256 + 256].reshape(D, 4, 64)[:, :, perm]
        wq01 = wqs[:, 0:2].reshape(D, 128)
        wq23 = wqs[:, 2:4].reshape(D, 128)
        wks = wk[:, m * 64 : m * 64 + 64][:, perm]
        wvs = wv[:, m * 64 : m * 64 + 64]
        wkv = np.concatenate([wvs, wks], axis=1)  # vT rows 0:64, k rows 64:128
        wqkv = np.ascontiguousarray(
            np.concatenate([wq01, wq23, wkv], axis=1)
        ).astype(BF16)
        wo01 = np.ascontiguousarray(wo[m * 256 : m * 256 + 128, :]).astype(BF16)
        wo23 = np.ascontiguousarray(wo[m * 256 + 128 : m * 256 + 256, :]).astype(
            BF16
        )
        in_maps.append(
            dict(xT=xT, wqkv=wqkv, wo01=wo01, wo23=wo23, cq=cq, sq=sq)
        )
    return in_maps


LAST_EXEC_NS = None


def _install_ntff_hook():
    """Provide antenv.axon_hooks (missing in some containers) so that
    run_bass_kernel_spmd(trace=True) can capture an NTFF profile."""
    import types

    try:
        import antenv.axon_hooks  # noqa: F401
        return True
    except ImportError:
        pass
    try:
        import antenv
        from trn_agent_boot.trn_boot import _ntff_profile_via_ctypes

        hook = _ntff_profile_via_ctypes("/opt/axon/libaxon_pjrt.so")
        if hook is None:
            return False
        mod = types.ModuleType("antenv.axon_hooks")
        mod._hook = hook
        mod.set_axon_ntff_profile_hook = lambda h: setattr(mod, "_hook", h)
        mod.get_axon_ntff_profile_hook = lambda: mod._hook
        sys.modules["antenv.axon_hooks"] = mod
        antenv.axon_hooks = mod
        return True
    except Exception:
        return False


def kernel(**inputs):
    global LAST_EXEC_NS
    from concourse import bass_utils

    in_maps = _prep_in_maps(inputs)
    nc = _build_bass()
    trace = bool(int(os.environ.get("KERNEL_TRACE", "0")))
    if trace:
        trace = _install_ntff_hook()
    res = bass_utils.run_bass_kernel_spmd(
        nc, in_maps, core_ids=list(range(8)), trace=trace
    )
    if trace and res.exec_time_ns is not None:
        LAST_EXEC_NS = res.exec_time_ns
    out = np.zeros((B, S, D), dtype=np.float32)
    for r in res.results:
        out += r["out"].astype(np.float32)
    return out


def time_device(reps=6, **inputs):
    """Wall-clock the sharded PJRT executable with device-resident inputs
    (fallback when NTFF profiling is unavailable; includes axon dispatch
    overhead)."""
    import jax
    from concourse import bass2jax
    import concourse.mybir as mybir
    import time as _time

    in_maps = _prep_in_maps(inputs)
    nc = _build_bass()
    bass2jax.install_neuronx_cc_hook()

    partition_name = (
        nc.partition_id_tensor.name if nc.partition_id_tensor else None
    )
    in_names, out_names, out_avals, zero_outs = [], [], [], []
    for alloc in nc.m.functions[0].allocations:
        if not isinstance(alloc, mybir.MemoryLocationSet):
            continue
        name = alloc.memorylocations[0].name
        if alloc.kind == "ExternalInput":
            if name != partition_name:
                in_names.append(name)
        elif alloc.kind == "ExternalOutput":
            out_names.append(name)
            shape = tuple(alloc.tensor_shape)
            dt = mybir.dt.np(alloc.dtype)
            out_avals.append(jax.core.ShapedArray(shape, dt))
            zero_outs.append(np.zeros(shape, dt))
    n_params = len(in_names)
    in_all = in_names + out_names
    if partition_name is not None:
        in_all = in_all + [partition_name]

    def _body(*args):
        operands = list(args)
        if partition_name is not None:
            operands.append(bass2jax.partition_id_tensor())
        outs = bass2jax._bass_exec_p.bind(
            *operands,
            out_avals=tuple(out_avals),
            in_names=tuple(in_all),
            out_names=tuple(out_names),
            lowering_input_output_aliases=(),
            sim_require_finite=True,
            sim_require_nnan=True,
            nc=nc,
        )
        return tuple(outs)

    devices = jax.devices()[:8]
    mesh = bass2jax.Mesh(np.asarray(devices), ("core",))
    spec = bass2jax.PartitionSpec("core")
    nin = n_params + len(out_names)
    f = jax.jit(
        bass2jax.shard_map(
            _body,
            mesh=mesh,
            in_specs=(spec,) * nin,
            out_specs=(spec,) * len(out_names),
            check_rep=False,
        )
    )
    concat_in = [
        np.concatenate([np.asarray(m[n]) for m in in_maps], axis=0)
        for n in in_names
    ]
    concat_zeros = [
        np.zeros((8 * z.shape[0], *z.shape[1:]), z.dtype) for z in zero_outs
    ]
    sharding = jax.sharding.NamedSharding(mesh, spec)
    dev_args = [jax.device_put(a, sharding) for a in concat_in + concat_zeros]
    r = f(*dev_args)
    jax.block_until_ready(r)
    best = None
    for _ in range(reps):
        t0 = _time.perf_counter()
        r = f(*dev_args)
        jax.block_until_ready(r)
        dt = _time.perf_counter() - t0
        best = dt if best is None else min(best, dt)
    return int(best * 1e9)
